# revision 1
# baseline (speedup 1.0000x reference)
"""Trainium2 Bass kernel for causal self-attention with cumulative-phase rotary
embedding (nn_CausalSelfAttention_64338610094602).

Sharding: 8 cores = 4 batches x 2 head-groups (tensor-parallel over heads).
Each core computes, for its (batch, 8-head group):
  omega/phi (replicated per batch), QKV projections, rotation + RMSNorm,
  causal attention (transposed-scores layout, max-free softmax), and a
  partial output projection. Host sums the two head-group partials per batch.

v4 design notes:
  - All big GEMM operands in bf16 (matmul rate keys off the moving operand;
    bf16 is 1 cycle/row at any N). PSUM accumulation stays fp32.
  - PSUM: one [128,512] ring of 4 banks (tag "q") shared by projections, v,
    scores, and P3 chains; [128,512] yps ring 2; [1,512] ring 2 for
    omega/ssq/denominators. Every accumulation chain owns a full bank
    (matmul start=True zeroes the whole bank).
  - Rotation: gamma folded into trig tiles (negated second half) so one
    full-width multiply + two swapped-half multiplies + one add write
    q_sb/k_sb directly in bf16.
  - RMSNorm: ACT Square -> M=1 PE colsum (deferred one site so the PE never
    waits on ACT) -> one Abs_reciprocal_sqrt -> GpSimd partition_broadcast.
    Same ACT table as Square/Exp path: no table swaps in steady state.
  - Causal mask folded into the PE accumulation: for diagonal score tiles,
    one extra matmul (tril stationary x -1e9 step moving) adds -1e9*count
    to masked entries, so exp() gives exact zeros.
  - 2c software-pipelined with lookahead-2 at key-tile granularity: scores
    for I and I-1 are issued before the consumers of I-2, giving the PE
    ~2us of cover work over the ACT Exp latency.
  - Softmax denominator reciprocal broadcast via GpSimd partition_broadcast;
    y spilled to DRAM in bf16 and streamed back in P3 (cb-outer loop with
    wo column-block and y tile prefetch).
"""
import math

import numpy as np
import ml_dtypes

import concourse.mybir as mybir
import concourse.tile as tile
from concourse import bacc
from concourse.bass_utils import run_bass_kernel_spmd

B, T, C = 4, 2048, 2048
H, D, DH = 16, 128, 64
HG = 8          # heads per core (head-group)
GD = HG * D     # group output dims = 1024
NT = T // 512   # 4 query blocks of 512
NCT = C // 128  # 16 contraction tiles
EPS = 1e-5
SCL = 1.0 / math.sqrt(D)
NEG = -1.0e9

dt = mybir.dt
AF = mybir.ActivationFunctionType
ALU = mybir.AluOpType

TWO_PI = 6.283185307179586
INV_2PI = 1.0 / TWO_PI
CW1 = float(np.float32(6.28125))
CW2 = float(np.float32(TWO_PI - 6.28125))
CW3 = float(TWO_PI - CW1 - float(np.float32(TWO_PI - 6.28125)))
MAGIC = 12582912.0  # 1.5 * 2^23: fp32 add/sub rounds to nearest int
HALF_PI = 1.5707963267948966
PI = 3.141592653589793

_CACHE = {}
DEBUG = False


def _build():
    f32, bf16 = dt.float32, dt.bfloat16
    nc = bacc.Bacc(None, target_bir_lowering=False)
    with tile.TileContext(nc) as tc:
        xt_d = nc.dram_tensor("xt", (C, T), bf16, kind="ExternalInput")
        wq_d = nc.dram_tensor("wq", (C, GD), bf16, kind="ExternalInput")
        wk_d = nc.dram_tensor("wk", (C, GD), bf16, kind="ExternalInput")
        wv_d = nc.dram_tensor("wv", (C, GD), bf16, kind="ExternalInput")
        wo_d = nc.dram_tensor("wo", (GD, C), bf16, kind="ExternalInput")
        womg2_d = nc.dram_tensor("womg2", (128, NCT * 128), bf16,
                                 kind="ExternalInput")
        b16_d = nc.dram_tensor("b16", (1, 1), f32, kind="ExternalInput")
        logf2_d = nc.dram_tensor("logf2", (128, 1), f32, kind="ExternalInput")
        gq_d = nc.dram_tensor("gq", (128, 1), f32, kind="ExternalInput")
        gqB_d = nc.dram_tensor("gqB", (128, 1), f32, kind="ExternalInput")
        gk_d = nc.dram_tensor("gk", (128, 1), f32, kind="ExternalInput")
        gkB_d = nc.dram_tensor("gkB", (128, 1), f32, kind="ExternalInput")
        trilA_d = nc.dram_tensor("trilA", (128, 128), bf16, kind="ExternalInput")
        maskB_d = nc.dram_tensor("maskB", (128, 4 * 512), bf16, kind="ExternalInput")
        ones128_d = nc.dram_tensor("ones128", (128, 128), bf16,
                                   kind="ExternalInput")
        out_d = nc.dram_tensor("out", (T, C), f32, kind="ExternalOutput")
        dbg = {}
        if DEBUG:
            dbg["q"] = nc.dram_tensor("dbg_q", (128, 2 * T), bf16,
                                      kind="ExternalOutput")
            dbg["k"] = nc.dram_tensor("dbg_k", (128, 2 * T), bf16,
                                      kind="ExternalOutput")
            dbg["v"] = nc.dram_tensor("dbg_v", (128, 16 * 256), bf16,
                                      kind="ExternalOutput")
            dbg["y"] = nc.dram_tensor("dbg_y", (128, HG * T), bf16,
                                      kind="ExternalOutput")

        with tc.tile_pool(name="const", bufs=1) as constp, \
             tc.tile_pool(name="dram", bufs=1, space="DRAM") as dramp, \
             tc.tile_pool(name="psp", bufs=1, space="PSUM") as psp:

            # ---- constants ----
            womg2 = constp.tile([128, NCT * 128], bf16)
            nc.sync.dma_start(womg2[:], womg2_d[:])
            b16t = constp.tile([1, 1], f32)
            nc.sync.dma_start(b16t[:], b16_d[:])
            logf2 = constp.tile([128, 1], f32)
            nc.sync.dma_start(logf2[:], logf2_d[:])
            gq = constp.tile([128, 1], f32)
            nc.sync.dma_start(gq[:], gq_d[:])
            gqB = constp.tile([128, 1], f32)
            nc.sync.dma_start(gqB[:], gqB_d[:])
            gk = constp.tile([128, 1], f32)
            nc.sync.dma_start(gk[:], gk_d[:])
            gkB = constp.tile([128, 1], f32)
            nc.sync.dma_start(gkB[:], gkB_d[:])
            trilA = constp.tile([128, 128], bf16)
            nc.sync.dma_start(trilA[:], trilA_d[:])
            maskB = constp.tile([128, 4 * 512], bf16)
            nc.sync.dma_start(maskB[:], maskB_d[:])
            ones128 = constp.tile([128, 128], bf16)
            nc.sync.dma_start(ones128[:], ones128_d[:])
            freq2 = constp.tile([128, 1], f32)
            nc.scalar.activation(freq2[:], logf2[:], AF.Exp)
            eps128 = constp.tile([128, 1], f32)
            nc.vector.memset(eps128[:], EPS)
            y_p3 = constp.tile([128, 2 * T], bf16)  # pair-3 y stays resident

            y_d = dramp.tile([128, HG * T], bf16)  # yT per head at col h*T

            with tc.tile_pool(name="big", bufs=1) as bigp, \
                 tc.tile_pool(name="xtp", bufs=1) as xtp, \
                 tc.tile_pool(name="wstp", bufs=1) as wstp, \
                 tc.tile_pool(name="scp", bufs=1) as scp, \
                 tc.tile_pool(name="rowp", bufs=1) as rowp:
                q_sb = bigp.tile([128, 2 * T], bf16)   # [D, hl*T + t]
                k_sb = bigp.tile([128, 2 * T], bf16)
                v_sb = bigp.tile([128, 16 * 256], bf16)  # key tile tt at tt*256
                trigA_q = bigp.tile([128, T], bf16)
                trigB_q = bigp.tile([128, T], bf16)
                trigA_k = bigp.tile([128, T], bf16)
                trigB_k = bigp.tile([128, T], bf16)
                _main(nc, tc, xt_d, wq_d, wk_d, wv_d,
                      xtp, wstp, scp, rowp, psp,
                      womg2, b16t, freq2, gq, gqB, gk, gkB, trilA, maskB,
                      ones128, eps128, q_sb, k_sb, v_sb, y_d, y_p3,
                      trigA_q, trigB_q, trigA_k, trigB_k, dbg)

            # ---- P3: out = y^T W_o (partial over heads); resident y and
            # wo column-blocks, cb-outer so the first chains start early ----
            with tc.tile_pool(name="p3w", bufs=1) as p3w, \
                 tc.tile_pool(name="p3o", bufs=1) as p3o:
                wo_slots = [None, None]

                def issue_wo(cb):
                    wob = p3w.tile([128, HG * 512], bf16, tag="wo", bufs=2,
                                   name=f"wo_{cb}")
                    for hh in range(HG):
                        nc.sync.dma_start(
                            wob[:, hh * 512:(hh + 1) * 512],
                            wo_d[hh * 128:(hh + 1) * 128,
                                 cb * 512:(cb + 1) * 512])
                    wo_slots[cb % 2] = wob

                issue_wo(0)
                yall = p3w.tile([128, 6 * T], bf16, name="yall")
                for c2 in range(12):
                    nc.sync.dma_start(yall[:, c2 * 1024:(c2 + 1) * 1024],
                                      y_d[:, c2 * 1024:(c2 + 1) * 1024])
                NTI = T // 128
                for cb in range(C // 512):
                    if cb + 1 < C // 512:
                        issue_wo(cb + 1)
                    wob = wo_slots[cb % 2]
                    for ti in range(NTI):
                        ops = psp.tile([128, 512], f32, tag="y", bufs=4,
                                       name=f"ops_{cb}_{ti}")
                        for hh in range(HG):
                            ysrc = (yall if hh < 6 else y_p3)
                            hb = hh if hh < 6 else hh - 6
                            nc.tensor.matmul(
                                ops[:],
                                ysrc[:, hb * T + ti * 128:hb * T + (ti + 1) * 128],
                                wob[:, hh * 512:(hh + 1) * 512],
                                start=(hh == 0), stop=(hh == HG - 1))
                        osb = p3o.tile([128, 512], f32, tag="osb", bufs=3)
                        nc.scalar.copy(osb[:], ops[:])
                        nc.sync.dma_start(
                            out_d[ti * 128:(ti + 1) * 128,
                                  cb * 512:(cb + 1) * 512],
                            osb[:])
    nc.compile()
    return nc


def _main(nc, tc, xt_d, wq_d, wk_d, wv_d,
          xtp, wstp, scp, rowp, psp,
          womg2, b16t, freq2, gq, gqB, gk, gkB, trilA, maskB,
          ones128, eps128, q_sb, k_sb, v_sb, y_d, y_p3,
          trigA_q, trigB_q, trigA_k, trigB_k, dbg):
    f32, bf16 = dt.float32, dt.bfloat16

    # x^T tiles, half-major DMA order so P1/2a can start early
    xts = xtp.tile([128, NCT * T], bf16)  # c-tile i at cols [i*T,(i+1)*T)

    def issue_xts():
        for half in range(2):
            for i in range(NCT):
                cs = half * 1024
                nc.sync.dma_start(
                    xts[:, i * T + cs:i * T + cs + 1024],
                    xt_d[i * 128:(i + 1) * 128, cs:cs + 1024])

    sites = [(pair, wi, hl) for pair in range(4) for wi in range(2)
             for hl in range(2)]
    wp_slots = [None, None]
    wvp_slots = [None]

    def issue_panel(si):
        pair, wi, hl = sites[si]
        h = pair * 2 + hl
        w_d = (wq_d, wk_d)[wi]
        wp = wstp.tile([128, NCT * 128], bf16, tag="wp", bufs=2,
                       name=f"wp_{si}")
        for i in range(NCT):
            nc.sync.dma_start(
                wp[:, i * 128:(i + 1) * 128],
                w_d[i * 128:(i + 1) * 128, h * 128:(h + 1) * 128])
        wp_slots[si % 2] = wp

    def issue_wvp(pair):
        wvp = wstp.tile([128, NCT * 256], bf16, tag="wvp", bufs=1,
                        name=f"wvp_{pair}")
        for i in range(NCT):
            nc.sync.dma_start(
                wvp[:, i * 256:(i + 1) * 256],
                wv_d[i * 128:(i + 1) * 128, pair * 256:(pair + 1) * 256])
        wvp_slots[0] = wvp

    issue_xts()
    issue_panel(0)
    issue_wvp(0)

    # ---- P1: omega -> phi -> trig (PE does only the omega matvecs) ----
    with tc.tile_pool(name="p1p", bufs=1) as p1p:
        omega = rowp.tile([1, T], f32, tag="om")
        for J in range(NT):
            omps = psp.tile([128, 512], f32, tag="y", bufs=4,
                            name=f"omps_{J}")
            for i in range(NCT):
                nc.tensor.matmul(
                    omps[:], womg2[:, i * 128:(i + 1) * 128],
                    xts[:, i * T + J * 512:i * T + J * 512 + 512],
                    start=(i == 0), stop=(i == NCT - 1))
            nc.scalar.activation(omega[:, J * 512:(J + 1) * 512],
                                 omps[0:1, :],
                                 AF.Sigmoid, scale=1.0 / 16.0, bias=b16t[:])
        HT = T // 2
        incl = rowp.tile([1, T], f32, tag="incl")
        nc.vector.tensor_tensor_scan(incl[:, 0:HT], omega[:, 0:HT],
                                     omega[:, 0:HT], 0.0,
                                     ALU.add, ALU.bypass)
        phi = rowp.tile([1, T], f32, tag="phi")
        nc.vector.tensor_sub(phi[:, 0:HT], incl[:, 0:HT], omega[:, 0:HT])
        nc.vector.tensor_tensor_scan(incl[:, HT:T], omega[:, HT:T],
                                     omega[:, HT:T], 0.0,
                                     ALU.add, ALU.bypass)
        nc.vector.tensor_scalar(incl[:, HT:T], incl[:, HT:T],
                                incl[:, HT - 1:HT], None, op0=ALU.add)
        nc.vector.tensor_sub(phi[:, HT:T], incl[:, HT:T], omega[:, HT:T])
        for J in range(NT):
            sl = slice(J * 512, (J + 1) * 512)
            phi2 = p1p.tile([128, 512], f32, tag="p1", bufs=4,
                            name=f"phi2_{J}")
            nc.gpsimd.partition_broadcast(phi2[:], phi[:, sl])
            ang = p1p.tile([128, 512], f32, tag="p1", bufs=4, name=f"ang_{J}")
            nc.vector.tensor_scalar(ang[:], phi2[:], freq2[:], None,
                                    op0=ALU.mult)
            mm = p1p.tile([128, 512], f32, tag="p1", bufs=4, name=f"mm_{J}")
            nc.vector.tensor_scalar(mm[:], ang[:], INV_2PI, MAGIC,
                                    op0=ALU.mult, op1=ALU.add)
            kk = p1p.tile([128, 512], f32, tag="p1", bufs=4, name=f"kk_{J}")
            nc.vector.tensor_scalar_add(kk[:], mm[:], -MAGIC)
            red = p1p.tile([128, 512], f32, tag="p1", bufs=4, name=f"red_{J}")
            nc.vector.cody_waite_cascade(red[:], ang[:], kk[:], CW1, CW2, CW3)
            red2 = p1p.tile([128, 512], f32, tag="p1", bufs=4,
                            name=f"red2_{J}")
            nc.vector.add_range_wrap(red2[:], red[:], HALF_PI, PI, TWO_PI)
            sinr = p1p.tile([128, 512], f32, tag="p1", bufs=4,
                            name=f"sinr_{J}")
            nc.scalar.activation(sinr[:], red[:], AF.Sin)
            cosr = p1p.tile([128, 512], f32, tag="p1", bufs=4,
                            name=f"cosr_{J}")
            nc.scalar.activation(cosr[:], red2[:], AF.Sin)
            nc.scalar.activation(trigA_q[:, sl], cosr[:], AF.Copy, scale=gq[:])
            nc.scalar.activation(trigB_q[:, sl], sinr[:], AF.Copy, scale=gqB[:])
            nc.scalar.activation(trigA_k[:, sl], cosr[:], AF.Copy, scale=gk[:])
            nc.scalar.activation(trigB_k[:, sl], sinr[:], AF.Copy, scale=gkB[:])

    # ---- P2 per pair ----
    pend_norm = [None]
    pend_epi = [None]
    ssq_queue = []  # FIFO of deferred rstd tails, emitted one site later

    def flush(pend):
        if pend[0] is not None:
            pend[0]()
            pend[0] = None

    for pair in range(4):
        wvp = wvp_slots[0]

        # --- 2a: q/k for both heads ---
        for wi in range(2):
            for hl in range(2):
                si = pair * 4 + wi * 2 + hl
                if si + 1 < len(sites):
                    issue_panel(si + 1)
                wp = wp_slots[si % 2]
                trigA = (trigA_q, trigA_k)[wi]
                trigB = (trigB_q, trigB_k)[wi]
                dest = (q_sb, k_sb)[wi]
                rnbs = []
                sqs = []
                for Jp in range(2):
                    # deferred rstd tails (one-site lag, one J-group per
                    # flush so the [1,512] psum ring never backs up)
                    while len(ssq_queue) > 1:
                        ssq_queue.pop(0)()
                    qps2 = psp.tile([128, 1024], f32, tag="s", bufs=2,
                                    name=f"qps2_{si}_{Jp}")
                    for i in range(NCT):
                        for Jh in range(2):
                            J = 2 * Jp + Jh
                            nc.tensor.matmul(
                                qps2[:, Jh * 512:(Jh + 1) * 512],
                                wp[:, i * 128:(i + 1) * 128],
                                xts[:, i * T + J * 512:i * T + J * 512 + 512],
                                start=(i == 0), stop=(i == NCT - 1))
                    for Jh in range(2):
                        J = 2 * Jp + Jh
                        qps = qps2[:, Jh * 512:(Jh + 1) * 512]
                        sl = slice(J * 512, (J + 1) * 512)
                        dcol = hl * T + J * 512
                        # rotation: A + swapped-half B, gamma folded in trig
                        A = scp.tile([128, 512], f32, tag="ra", bufs=3,
                                     name=f"A_{si}_{J}")
                        nc.vector.tensor_tensor(A[:], qps, trigA[:, sl],
                                                op=ALU.mult)
                        Bt = scp.tile([128, 512], f32, tag="rb", bufs=3,
                                      name=f"Bt_{si}_{J}")
                        nc.vector.tensor_tensor(
                            Bt[0:DH, :],
                            qps2[DH:128, Jh * 512:(Jh + 1) * 512],
                            trigB[0:DH, sl], op=ALU.mult)
                        nc.vector.tensor_tensor(
                            Bt[DH:128, :],
                            qps2[0:DH, Jh * 512:(Jh + 1) * 512],
                            trigB[DH:128, sl], op=ALU.mult)
                        nc.vector.tensor_add(
                            dest[:, dcol:dcol + 512], A[:], Bt[:])
                        # sum-of-squares path (rotation preserves norms)
                        sq = scp.tile([128, 512], bf16, tag="sq", bufs=6,
                                      name=f"sq_{si}_{J}")
                        nc.scalar.activation(sq[:], qps, AF.Square)
                        sqs.append((J, sq))

                    def ssq_tail(sqs=tuple(sqs[-2:]), rnbs=rnbs, si=si):
                        for J, sq in sqs:
                            ssqps = psp.tile([128, 512], f32, tag="y", bufs=4,
                                             name=f"ssq_{si}_{J}")
                            nc.tensor.matmul(ssqps[:], ones128[:], sq[:],
                                             start=True, stop=True)
                            rnb = scp.tile([128, 512], bf16, tag="rnb",
                                           bufs=4, name=f"rnb_{si}_{J}")
                            nc.scalar.activation(rnb[:], ssqps[:],
                                                 AF.Abs_reciprocal_sqrt,
                                                 scale=1.0 / 128.0,
                                                 bias=eps128[:])
                            rnbs.append((J, rnb))
                    ssq_queue.append(ssq_tail)
                flush(pend_norm)

                def norm(dest=dest, hl=hl, rnbs=rnbs):
                    for J, rnb in rnbs:
                        dcol = hl * T + J * 512
                        nc.vector.tensor_tensor(
                            dest[:, dcol:dcol + 512],
                            dest[:, dcol:dcol + 512],
                            rnb[:], op=ALU.mult)
                pend_norm[0] = norm
        if dbg and pair == 0:
            nc.sync.dma_start(dbg["q"][:], q_sb[:])
            nc.sync.dma_start(dbg["k"][:], k_sb[:])

        # --- 2b: v for both heads; each accumulation chain owns a full PSUM
        # bank: quarters 0 and 2 of two [128,1024] tiles = 4 banks ---
        for tq in range(4):
            vps = []
            for q4 in range(2):
                vps.append(psp.tile([128, 1024], f32, tag="s", bufs=2,
                                    name=f"vps_{pair}_{tq}_{q4}"))
            for q4 in range(2):
                for i in range(NCT):
                    for t2 in range(2):
                        t = q4 * 2 + t2
                        tt = tq * 4 + t
                        nc.tensor.matmul(
                            vps[q4][:, t2 * 512:t2 * 512 + 256],
                            xts[:, i * T + tt * 128:i * T + (tt + 1) * 128],
                            wvp[:, i * 256:(i + 1) * 256],
                            start=(i == 0), stop=(i == NCT - 1))
            for t in range(4):
                tt = tq * 4 + t
                nc.vector.tensor_copy(
                    v_sb[:, tt * 256:(tt + 1) * 256],
                    vps[t // 2][:, (t % 2) * 512:(t % 2) * 512 + 256])
            if tq == 0:
                while ssq_queue:
                    ssq_queue.pop(0)()
                flush(pend_norm)
        if pair + 1 < 4:
            issue_wvp(pair + 1)
        if dbg and pair == 0:
            nc.sync.dma_start(dbg["v"][:], v_sb[:])

        # --- 2c: attention, software-pipelined lookahead-2 ---
        for hl in range(2):
            h = pair * 2 + hl
            for J in range(NT):
                nI = 4 * J + 4
                yps = psp.tile([128, 512], f32, tag="y", bufs=4,
                               name=f"yps_{pair}_{hl}_{J}")
                dps = psp.tile([128, 512], f32, tag="y", bufs=4,
                               name=f"dps_{pair}_{hl}_{J}")
                exq = []

                def consume(ex2, I0, yps=yps, dps=dps, hl=hl, nI=nI):
                    for half2 in range(2):
                        I = I0 + half2
                        exsl = ex2[:, half2 * 512:(half2 + 1) * 512]
                        nc.tensor.matmul(
                            yps[:],
                            v_sb[:, I * 256 + hl * 128:I * 256 + hl * 128 + 128],
                            exsl, start=(I == 0), stop=(I == nI - 1))
                        nc.tensor.matmul(
                            dps[:], ones128[:], exsl,
                            start=(I == 0), stop=(I == nI - 1))

                for Ip in range(nI // 2):
                    sps2 = psp.tile([128, 1024], f32, tag="s", bufs=2,
                                    name=f"sps_{pair}_{hl}_{J}_{Ip}")
                    for half2 in range(2):
                        I = 2 * Ip + half2
                        diag = I >= 4 * J
                        osl = sps2[:, half2 * 512:(half2 + 1) * 512]
                        nc.tensor.matmul(
                            osl,
                            k_sb[:, hl * T + I * 128:hl * T + (I + 1) * 128],
                            q_sb[:, hl * T + J * 512:hl * T + (J + 1) * 512],
                            start=True, stop=(not diag))
                        if diag:
                            r = I - 4 * J
                            nc.tensor.matmul(
                                osl, trilA[:], maskB[:, r * 512:(r + 1) * 512],
                                start=False, stop=True)
                    ex2 = scp.tile([128, 1024], bf16, tag="ex", bufs=4,
                                   name=f"ex_{pair}_{hl}_{J}_{Ip}")
                    nc.scalar.activation(ex2[:], sps2[:], AF.Exp, scale=SCL)
                    exq.append((ex2, 2 * Ip))
                    if len(exq) > 2:
                        consume(*exq.pop(0))
                    if Ip == 0:
                        flush(pend_epi)
                while exq:
                    consume(*exq.pop(0))

                def epilogue(yps=yps, dps=dps, h=h, J=J, pair=pair):
                    rb = scp.tile([128, 512], f32, tag="rbc", bufs=2,
                                  name=f"rb_{h}_{J}")
                    nc.vector.reciprocal_approx_fast(out=rb[:], in_=dps[:])
                    if pair == 3:
                        dst = y_p3[:, (h - 6) * T + J * 512:
                                   (h - 6) * T + (J + 1) * 512]
                        nc.vector.tensor_tensor(dst, yps[:], rb[:],
                                                op=ALU.mult)
                        return
                    yt = scp.tile([128, 512], bf16, tag="yt", bufs=2,
                                  name=f"yt_{h}_{J}")
                    nc.vector.tensor_tensor(yt[:], yps[:], rb[:], op=ALU.mult)
                    nc.sync.dma_start(
                        y_d[:, h * T + J * 512:h * T + (J + 1) * 512], yt[:])
                    if dbg:
                        nc.sync.dma_start(
                            dbg["y"][:, h * T + J * 512:h * T + (J + 1) * 512],
                            yt[:])
                pend_epi[0] = epilogue
        flush(pend_epi)


def _host_prep(inputs):
    bf = ml_dtypes.bfloat16
    x = np.asarray(inputs["x"], dtype=np.float32)
    Wq = np.asarray(inputs["Wq"], dtype=np.float32)
    Wk = np.asarray(inputs["Wk"], dtype=np.float32)
    Wv = np.asarray(inputs["Wv"], dtype=np.float32)
    Wo = np.asarray(inputs["Wo"], dtype=np.float32)
    w_omega = np.asarray(inputs["w_omega"], dtype=np.float32)
    b_omega = np.asarray(inputs["b_omega"], dtype=np.float32)
    log_freq = np.asarray(inputs["log_freq"], dtype=np.float32)
    q_gamma = np.asarray(inputs["q_gamma"], dtype=np.float32)
    k_gamma = np.asarray(inputs["k_gamma"], dtype=np.float32)

    womg = w_omega.reshape(NCT, 128).T.astype(np.float32)
    # replicated across output rows: womg2[:, i*128+c] = w_omega[i*128+:] col c
    womg2 = np.repeat(womg.T[:, :, None], 128, axis=2)  # [i, 128k, 128c]
    womg2 = womg2.transpose(1, 0, 2).reshape(128, NCT * 128).astype(bf)
    b16 = (b_omega / 16.0).reshape(1, 1).astype(np.float32)
    logf2 = np.concatenate([log_freq, log_freq]).reshape(128, 1)
    gqv = q_gamma.reshape(128, 1).astype(np.float32)
    gqB = np.concatenate([q_gamma[:DH], -q_gamma[DH:]]).reshape(128, 1)
    gkv = k_gamma.reshape(128, 1).astype(np.float32)
    gkB = np.concatenate([k_gamma[:DH], -k_gamma[DH:]]).reshape(128, 1)
    kk = np.arange(128)
    trilA = (kk[:, None] <= kk[None, :]).astype(bf)  # [k, p] = (k <= p)
    p = np.arange(128)[:, None]
    c = np.arange(512)[None, :]
    maskB = np.concatenate(
        [(NEG * ((p + r * 128) > c)).astype(np.float32) for r in range(4)],
        axis=1).astype(bf)
    ones128 = np.ones((128, 128), dtype=bf)

    in_maps = []
    for core in range(8):
        b, g = core // 2, core % 2
        in_maps.append({
            "xt": np.ascontiguousarray(x[b].T).astype(bf),
            "wq": np.ascontiguousarray(Wq[g * GD:(g + 1) * GD, :].T).astype(bf),
            "wk": np.ascontiguousarray(Wk[g * GD:(g + 1) * GD, :].T).astype(bf),
            "wv": np.ascontiguousarray(Wv[g * GD:(g + 1) * GD, :].T).astype(bf),
            "wo": np.ascontiguousarray(Wo[:, g * GD:(g + 1) * GD].T).astype(bf),
            "womg2": womg2, "b16": b16,
            "logf2": logf2.astype(np.float32),
            "gq": gqv, "gqB": gqB.astype(np.float32),
            "gk": gkv, "gkB": gkB.astype(np.float32),
            "trilA": trilA, "maskB": maskB, "ones128": ones128,
        })
    return in_maps


def kernel(**inputs) -> np.ndarray:
    if "nc" not in _CACHE:
        _CACHE["nc"] = _build()
    nc = _CACHE["nc"]
    in_maps = _host_prep(inputs)
    res = run_bass_kernel_spmd(nc, in_maps, core_ids=list(range(8)))
    out = np.empty((B, T, C), dtype=np.float32)
    for b in range(B):
        out[b] = res.results[2 * b]["out"] + res.results[2 * b + 1]["out"]
    return out



# revision 6
# speedup vs baseline: 1.0185x; 1.0185x over previous
"""Trainium2 Bass kernel for causal self-attention with cumulative-phase rotary
embedding (nn_CausalSelfAttention_64338610094602).

Sharding: 8 cores = 4 batches x 2 head-groups (tensor-parallel over heads).
Each core computes, for its (batch, 8-head group):
  omega/phi (replicated per batch), QKV projections, rotation + RMSNorm,
  causal attention (transposed-scores layout, max-free softmax), and a
  partial output projection. Host sums the two head-group partials per batch.

v5 design notes (vs v4's per-pair phases):
  - All projections first (P1 omega/trig, P2 all 4 pairs' q/k/v), then one
    flat attention pipeline over all 32 (head, J) block-rows, then P4.
    The PE instruction stream never alternates sections, which avoids both
    the per-row ACT-latency bubbles and the p-state ramp (PE runs at 1.2GHz
    for 3us after any idle gap, 2.4GHz only when continuously busy).
  - q/k (all 8 heads, post-norm, bf16) spill to DRAM during P2 and stream
    back per-head in P3 (SBUF cannot hold 8 heads of q+k next to xts);
    v and y stay SBUF-resident for all heads (no y round-trip).
  - Softmax denominator: each ex2 [128,1024] tile is folded to [128,512]
    on DVE (bf16 add of the two key-tile halves) and the PE ones-matmul
    runs on the folded tile -- half the PE columns of v4's dps.
  - Rotation sign baked into the frequency vector (rows 64:128 negative)
    so trig tiles are written straight out of ACT Sin; gamma applied in the
    RMSNorm multiply (scalar_tensor_tensor) instead of folded into trig.
  - Causal mask folded into the PE score accumulation (trilA x maskB adds
    -1e9*count on diagonal tiles) as in v4.
  - All 4 Wo column blocks prefetched into SBUF during P3; P4 reads y_sb
    directly, so the P3->P4 transition has no DMA wait.
"""
import math

import numpy as np
import ml_dtypes

import concourse.mybir as mybir
import concourse.tile as tile
from concourse import bacc
from concourse.bass_utils import run_bass_kernel_spmd

B, T, C = 4, 2048, 2048
H, D, DH = 16, 128, 64
HG = 8          # heads per core (head-group)
GD = HG * D     # group output dims = 1024
NT = T // 512   # 4 query blocks of 512
NCT = C // 128  # 16 contraction tiles
EPS = 1e-5
SCL = 1.0 / math.sqrt(D)
NEG = -1.0e9

dt = mybir.dt
AF = mybir.ActivationFunctionType
ALU = mybir.AluOpType

TWO_PI = 6.283185307179586
INV_2PI = 1.0 / TWO_PI
CW1 = float(np.float32(6.28125))
CW2 = float(np.float32(TWO_PI - 6.28125))
CW3 = float(TWO_PI - CW1 - float(np.float32(TWO_PI - 6.28125)))
MAGIC = 12582912.0  # 1.5 * 2^23: fp32 add/sub rounds to nearest int
HALF_PI = 1.5707963267948966
PI = 3.141592653589793

_CACHE = {}


def _build():
    f32, bf16 = dt.float32, dt.bfloat16
    nc = bacc.Bacc(None, target_bir_lowering=False)
    with tile.TileContext(nc) as tc:
        xt_d = nc.dram_tensor("xt", (C, T), bf16, kind="ExternalInput")
        wq_d = nc.dram_tensor("wq", (C, GD), bf16, kind="ExternalInput")
        wk_d = nc.dram_tensor("wk", (C, GD), bf16, kind="ExternalInput")
        wv_d = nc.dram_tensor("wv", (C, GD), bf16, kind="ExternalInput")
        wo_d = nc.dram_tensor("wo", (GD, C), bf16, kind="ExternalInput")
        womg2_d = nc.dram_tensor("womg2", (128, NCT * 128), bf16,
                                 kind="ExternalInput")
        b16_d = nc.dram_tensor("b16", (1, 1), f32, kind="ExternalInput")
        freqs_d = nc.dram_tensor("freqs", (128, 1), f32, kind="ExternalInput")
        gq_d = nc.dram_tensor("gq", (128, 1), f32, kind="ExternalInput")
        gk_d = nc.dram_tensor("gk", (128, 1), f32, kind="ExternalInput")
        trilA_d = nc.dram_tensor("trilA", (128, 128), bf16, kind="ExternalInput")
        maskB_d = nc.dram_tensor("maskB", (128, 4 * 512), bf16, kind="ExternalInput")
        ones128_d = nc.dram_tensor("ones128", (128, 128), bf16,
                                   kind="ExternalInput")
        out_d = nc.dram_tensor("out", (T, C), f32, kind="ExternalOutput")

        with tc.tile_pool(name="const", bufs=1) as constp, \
             tc.tile_pool(name="dram", bufs=1, space="DRAM") as dramp, \
             tc.tile_pool(name="core", bufs=1) as corep, \
             tc.tile_pool(name="psp", bufs=1, space="PSUM") as psp:

            # ---- constants ----
            b16t = constp.tile([1, 1], f32)
            nc.sync.dma_start(b16t[:], b16_d[:])
            freqs = constp.tile([128, 1], f32)
            nc.sync.dma_start(freqs[:], freqs_d[:])
            gq = constp.tile([128, 1], f32)
            nc.sync.dma_start(gq[:], gq_d[:])
            gk = constp.tile([128, 1], f32)
            nc.sync.dma_start(gk[:], gk_d[:])
            trilA = constp.tile([128, 128], bf16)
            nc.sync.dma_start(trilA[:], trilA_d[:])
            maskB = constp.tile([128, 4 * 512], bf16)
            nc.sync.dma_start(maskB[:], maskB_d[:])
            ones128 = constp.tile([128, 128], bf16)
            nc.sync.dma_start(ones128[:], ones128_d[:])
            eps128 = constp.tile([128, 1], f32)
            nc.vector.memset(eps128[:], EPS)

            # all-heads v and y stay resident; q/k spill to DRAM
            v_sb = corep.tile([128, 4 * 16 * 256], bf16)  # (pair*16+tt)*256
            y_sb = corep.tile([128, HG * T], bf16)        # yT per head at h*T
            qk_d = dramp.tile([128, 2 * HG * T], bf16)    # q at h*T, k at (8+h)*T

            with tc.tile_pool(name="xtp", bufs=1) as xtp, \
                 tc.tile_pool(name="wstp", bufs=1) as wstp, \
                 tc.tile_pool(name="trigp", bufs=1) as trigp:
                trigA = trigp.tile([128, T], bf16)
                trigB = trigp.tile([128, T], bf16)
                _proj(nc, tc, xt_d, wq_d, wk_d, wv_d, womg2_d,
                      xtp, wstp, psp,
                      b16t, freqs, gq, gk, ones128, eps128,
                      trigA, trigB, v_sb, qk_d)

            with tc.tile_pool(name="qkp", bufs=1) as qkp, \
                 tc.tile_pool(name="attp", bufs=1) as attp, \
                 tc.tile_pool(name="p4w", bufs=1) as p4w, \
                 tc.tile_pool(name="p4o", bufs=1) as p4o:
                # stream q/k per head (ring 2), prefetch all wo blocks
                qh_slots = [None, None]

                def fetch_head(h):
                    qh = qkp.tile([128, T], bf16, tag="qh", bufs=2,
                                  name=f"qh_{h}")
                    kh = qkp.tile([128, T], bf16, tag="kh", bufs=2,
                                  name=f"kh_{h}")
                    nc.sync.dma_start(qh[:], qk_d[:, h * T:(h + 1) * T])
                    nc.sync.dma_start(kh[:], qk_d[:, (HG + h) * T:(HG + h + 1) * T])
                    qh_slots[h % 2] = (qh, kh)

                fetch_head(0)
                fetch_head(1)
                wo_all = p4w.tile([128, 4 * HG * 512], bf16)  # (cb*8+hh)*512
                for cb in range(4):
                    for hh in range(HG):
                        nc.sync.dma_start(
                            wo_all[:, (cb * 8 + hh) * 512:(cb * 8 + hh + 1) * 512],
                            wo_d[hh * 128:(hh + 1) * 128,
                                 cb * 512:(cb + 1) * 512])

                _attention(nc, tc, attp, psp, qh_slots, fetch_head,
                           trilA, maskB, ones128, v_sb, y_sb)

                # ---- P4: out = y^T W_o (partial over heads) ----
                for ti in range(T // 128):
                    for cb in range(4):
                        ops = psp.tile([128, 512], f32, tag="y", bufs=4,
                                       name=f"ops_{ti}_{cb}")
                        for hh in range(HG):
                            nc.tensor.matmul(
                                ops[:],
                                y_sb[:, hh * T + ti * 128:hh * T + (ti + 1) * 128],
                                wo_all[:, (cb * 8 + hh) * 512:(cb * 8 + hh + 1) * 512],
                                start=(hh == 0), stop=(hh == HG - 1))
                        osb = p4o.tile([128, 512], f32, tag="osb", bufs=3)
                        nc.scalar.copy(osb[:], ops[:])
                        nc.sync.dma_start(
                            out_d[ti * 128:(ti + 1) * 128,
                                  cb * 512:(cb + 1) * 512],
                            osb[:])
    nc.compile()
    return nc


def _proj(nc, tc, xt_d, wq_d, wk_d, wv_d, womg2_d,
          xtp, wstp, psp,
          b16t, freqs, gq, gk, ones128, eps128,
          trigA, trigB, v_sb, qk_d):
    f32, bf16 = dt.float32, dt.bfloat16

    sites = [(pair, wi, hl) for pair in range(4) for wi in range(2)
             for hl in range(2)]
    wp_slots = [None, None]
    wvp_slots = [None]

    def issue_panel(si):
        pair, wi, hl = sites[si]
        h = pair * 2 + hl
        w_d = (wq_d, wk_d)[wi]
        wp = wstp.tile([128, NCT * 128], bf16, tag="wp", bufs=2,
                       name=f"wp_{si}")
        for i in range(NCT):
            nc.sync.dma_start(
                wp[:, i * 128:(i + 1) * 128],
                w_d[i * 128:(i + 1) * 128, h * 128:(h + 1) * 128])
        wp_slots[si % 2] = wp

    def issue_wvp(pair):
        wvp = wstp.tile([128, NCT * 256], bf16, tag="wvp", bufs=1,
                        name=f"wvp_{pair}")
        for i in range(NCT):
            nc.sync.dma_start(
                wvp[:, i * 256:(i + 1) * 256],
                wv_d[i * 128:(i + 1) * 128, pair * 256:(pair + 1) * 256])
        wvp_slots[0] = wvp

    # ---- P1: omega -> phi -> trig (pools closed before P2's scratch) ----
    with tc.tile_pool(name="p1p", bufs=1) as p1p, \
         tc.tile_pool(name="rowp", bufs=1) as rowp:
        womg2 = p1p.tile([128, NCT * 128], bf16, name="womg2")
        nc.sync.dma_start(womg2[:], womg2_d[:])
        issue_panel(0)
        # x^T tiles, quarter-major so P1 J-chains finish as quarters arrive
        xts = xtp.tile([128, NCT * T], bf16)  # c-tile i at [i*T,(i+1)*T)
        for J in range(NT):
            for i in range(NCT):
                cs = J * 512
                nc.sync.dma_start(
                    xts[:, i * T + cs:i * T + cs + 512],
                    xt_d[i * 128:(i + 1) * 128, cs:cs + 512])
        issue_wvp(0)

        omega = rowp.tile([1, T], f32, tag="om")
        for J in range(NT):
            omps = psp.tile([128, 512], f32, tag="y", bufs=4,
                            name=f"omps_{J}")
            for i in range(NCT):
                nc.tensor.matmul(
                    omps[:], womg2[:, i * 128:(i + 1) * 128],
                    xts[:, i * T + J * 512:i * T + J * 512 + 512],
                    start=(i == 0), stop=(i == NCT - 1))
            nc.scalar.activation(omega[:, J * 512:(J + 1) * 512],
                                 omps[0:1, :],
                                 AF.Sigmoid, scale=1.0 / 16.0, bias=b16t[:])
        HT = T // 2
        # phi computed in-place in incl (= cumsum - omega)
        incl = rowp.tile([1, T], f32, tag="incl")
        nc.vector.tensor_tensor_scan(incl[:, 0:HT], omega[:, 0:HT],
                                     omega[:, 0:HT], 0.0,
                                     ALU.add, ALU.bypass)
        nc.vector.tensor_tensor_scan(incl[:, HT:T], omega[:, HT:T],
                                     omega[:, HT:T], 0.0,
                                     ALU.add, ALU.bypass)
        nc.vector.tensor_scalar(incl[:, HT:T], incl[:, HT:T],
                                incl[:, HT - 1:HT], None, op0=ALU.add)
        phi = incl
        nc.vector.tensor_sub(phi[:], incl[:], omega[:])
        for J in range(NT):
            sl = slice(J * 512, (J + 1) * 512)
            phi2 = p1p.tile([128, 512], f32, tag="p1", bufs=4,
                            name=f"phi2_{J}")
            nc.gpsimd.partition_broadcast(phi2[:], phi[:, sl])
            ang = p1p.tile([128, 512], f32, tag="p1", bufs=4, name=f"ang_{J}")
            # rows 64:128 of freqs are negated: sin rows come out negated,
            # cos rows unchanged (even), which is the rotation's sign layout
            nc.vector.tensor_scalar(ang[:], phi2[:], freqs[:], None,
                                    op0=ALU.mult)
            mm = p1p.tile([128, 512], f32, tag="p1", bufs=4, name=f"mm_{J}")
            nc.vector.tensor_scalar(mm[:], ang[:], INV_2PI, MAGIC,
                                    op0=ALU.mult, op1=ALU.add)
            kk = p1p.tile([128, 512], f32, tag="p1", bufs=4, name=f"kk_{J}")
            nc.vector.tensor_scalar_add(kk[:], mm[:], -MAGIC)
            red = p1p.tile([128, 512], f32, tag="p1", bufs=4, name=f"red_{J}")
            nc.vector.cody_waite_cascade(red[:], ang[:], kk[:], CW1, CW2, CW3)
            red2 = p1p.tile([128, 512], f32, tag="p1", bufs=4,
                            name=f"red2_{J}")
            nc.vector.add_range_wrap(red2[:], red[:], HALF_PI, PI, TWO_PI)
            nc.scalar.activation(trigB[:, sl], red[:], AF.Sin)
            nc.scalar.activation(trigA[:, sl], red2[:], AF.Sin)

    # ---- P2: q/k/v for all pairs; q/k rotated+normed then spilled ----
    pend_norm = [None]
    pend_tail = [None]

    def flush(pend):
        if pend[0] is not None:
            pend[0]()
            pend[0] = None

    with tc.tile_pool(name="scp", bufs=1) as scp:
        for pair in range(4):
            wvp = wvp_slots[0]

            for wi in range(2):
                for hl in range(2):
                    si = pair * 4 + wi * 2 + hl
                    if si + 1 < len(sites):
                        issue_panel(si + 1)
                    wp = wp_slots[si % 2]
                    h = pair * 2 + hl
                    dcol0 = (wi * HG + h) * T  # q at h*T, k at (8+h)*T
                    g = (gq, gk)[wi]
                    qsite = scp.tile([128, T], bf16, tag="qk", bufs=2,
                                     name=f"qsite_{si}")
                    sqs = []
                    for Jp in range(2):
                        qps2 = psp.tile([128, 1024], f32, tag="s", bufs=2,
                                        name=f"qps2_{si}_{Jp}")
                        for i in range(NCT):
                            for Jh in range(2):
                                J = 2 * Jp + Jh
                                nc.tensor.matmul(
                                    qps2[:, Jh * 512:(Jh + 1) * 512],
                                    wp[:, i * 128:(i + 1) * 128],
                                    xts[:, i * T + J * 512:i * T + J * 512 + 512],
                                    start=(i == 0), stop=(i == NCT - 1))
                        # flush prev site's ssq tail mid-stream so its rnb
                        # is ready before this site's norm
                        if Jp == 1:
                            flush(pend_tail)
                        for Jh in range(2):
                            J = 2 * Jp + Jh
                            qps = qps2[:, Jh * 512:(Jh + 1) * 512]
                            sl = slice(J * 512, (J + 1) * 512)
                            # rotation: cos part straight into qsite, then
                            # += swapped-half sin part (sign baked in trigB)
                            nc.vector.tensor_tensor(qsite[:, sl], qps,
                                                    trigA[:, sl], op=ALU.mult)
                            Bt = scp.tile([128, 512], f32, tag="rb", bufs=2,
                                          name=f"Bt_{si}_{J}")
                            nc.vector.tensor_tensor(
                                Bt[0:DH, :],
                                qps2[DH:128, Jh * 512:(Jh + 1) * 512],
                                trigB[0:DH, sl], op=ALU.mult)
                            nc.vector.tensor_tensor(
                                Bt[DH:128, :],
                                qps2[0:DH, Jh * 512:(Jh + 1) * 512],
                                trigB[DH:128, sl], op=ALU.mult)
                            nc.vector.tensor_add(
                                qsite[:, sl], qsite[:, sl], Bt[:])
                            # sum-of-squares (rotation preserves norms)
                            sq = scp.tile([128, 512], bf16, tag="sq", bufs=6,
                                          name=f"sq_{si}_{J}")
                            nc.scalar.activation(sq[:], qps, AF.Square)
                            sqs.append((J, sq))
                    flush(pend_norm)

                    def tail(sqs=tuple(sqs), si=si, qsite=qsite, g=g,
                             dcol0=dcol0, pend_norm=pend_norm):
                        rnbs = []
                        for J, sq in sqs:
                            ssqps = psp.tile([128, 512], f32, tag="y", bufs=4,
                                             name=f"ssq_{si}_{J}")
                            nc.tensor.matmul(ssqps[:], ones128[:], sq[:],
                                             start=True, stop=True)
                            rnb = scp.tile([128, 512], bf16, tag="rnb",
                                           bufs=4, name=f"rnb_{si}_{J}")
                            nc.scalar.activation(rnb[:], ssqps[:],
                                                 AF.Abs_reciprocal_sqrt,
                                                 scale=1.0 / 128.0,
                                                 bias=eps128[:])
                            rnbs.append((J, rnb))

                        def norm():
                            for J, rnb in rnbs:
                                sl = slice(J * 512, (J + 1) * 512)
                                nc.vector.scalar_tensor_tensor(
                                    qsite[:, sl], qsite[:, sl], g[:], rnb[:],
                                    op0=ALU.mult, op1=ALU.mult)
                            nc.sync.dma_start(qk_d[:, dcol0:dcol0 + T],
                                              qsite[:])
                        pend_norm[0] = norm
                    pend_tail[0] = tail

            # --- v for both heads of the pair ---
            vbase = pair * 16 * 256
            for tq in range(4):
                vps = []
                for q4 in range(2):
                    vps.append(psp.tile([128, 1024], f32, tag="s", bufs=2,
                                        name=f"vps_{pair}_{tq}_{q4}"))
                for q4 in range(2):
                    for i in range(NCT):
                        for t2 in range(2):
                            t = q4 * 2 + t2
                            tt = tq * 4 + t
                            nc.tensor.matmul(
                                vps[q4][:, t2 * 512:t2 * 512 + 256],
                                xts[:, i * T + tt * 128:i * T + (tt + 1) * 128],
                                wvp[:, i * 256:(i + 1) * 256],
                                start=(i == 0), stop=(i == NCT - 1))
                for t in range(4):
                    tt = tq * 4 + t
                    nc.vector.tensor_copy(
                        v_sb[:, vbase + tt * 256:vbase + (tt + 1) * 256],
                        vps[t // 2][:, (t % 2) * 512:(t % 2) * 512 + 256])
                if tq == 0:
                    flush(pend_tail)
                    flush(pend_norm)
            if pair + 1 < 4:
                issue_wvp(pair + 1)
        flush(pend_tail)
        flush(pend_norm)


def _attention(nc, tc, attp, psp, qh_slots, fetch_head,
               trilA, maskB, ones128, v_sb, y_sb):
    """Flat software pipeline over all (h, J) block-rows at Ip granularity.

    Per task (h, J, Ip): scores for key-tile pair Ip into a [128,1024] PSUM
    tile (mask folded in on diagonal tiles), ACT Exp -> ex2 bf16, DVE fold
    of the two halves for the denominator.  Consumption lags 2 tasks: yps
    matmuls per half plus one dps matmul on the folded tile.  Row epilogue
    (reciprocal + y write) runs on DVE.
    """
    f32, bf16 = dt.float32, dt.bfloat16
    tasks = []
    for h in range(HG):
        for J in range(NT):
            for Ip in range(2 * J + 2):
                tasks.append((h, J, Ip))

    state = {}  # (h, J) -> (yps, dps)
    inflight = []

    def issue(t):
        h, J, Ip = t
        if J == 0 and Ip == 0 and 1 <= h < HG - 1:
            # heads 0/1 are prefetched before the pipeline; ring slot h-1
            # frees once all of head h-1's scores have issued
            fetch_head(h + 1)
        qh, kh = qh_slots[h % 2]
        sps2 = psp.tile([128, 1024], f32, tag="s", bufs=2,
                        name=f"sps_{h}_{J}_{Ip}")
        for half in range(2):
            I = 2 * Ip + half
            diag = I >= 4 * J
            osl = sps2[:, half * 512:(half + 1) * 512]
            nc.tensor.matmul(
                osl,
                kh[:, I * 128:(I + 1) * 128],
                qh[:, J * 512:(J + 1) * 512],
                start=True, stop=(not diag))
            if diag:
                r = I - 4 * J
                nc.tensor.matmul(
                    osl, trilA[:], maskB[:, r * 512:(r + 1) * 512],
                    start=False, stop=True)
        ex2 = attp.tile([128, 1024], bf16, tag="ex", bufs=4,
                        name=f"ex_{h}_{J}_{Ip}")
        nc.scalar.activation(ex2[:], sps2[:], AF.Exp, scale=SCL)
        fold = attp.tile([128, 512], bf16, tag="fold", bufs=4,
                         name=f"fold_{h}_{J}_{Ip}")
        nc.vector.tensor_add(fold[:], ex2[:, 0:512], ex2[:, 512:1024])
        return (t, ex2, fold)

    def consume(item):
        t, ex2, fold = item
        h, J, Ip = t
        nI = 4 * J + 4
        nIp = 2 * J + 2
        if Ip == 0:
            yps = psp.tile([128, 512], f32, tag="y", bufs=4,
                           name=f"yps_{h}_{J}")
            dps = psp.tile([128, 512], f32, tag="y", bufs=4,
                           name=f"dps_{h}_{J}")
            state[(h, J)] = (yps, dps)
        yps, dps = state[(h, J)]
        vbase = (h // 2) * 16 * 256
        hoff = (h % 2) * 128
        for half in range(2):
            I = 2 * Ip + half
            nc.tensor.matmul(
                yps[:],
                v_sb[:, vbase + I * 256 + hoff:vbase + I * 256 + hoff + 128],
                ex2[:, half * 512:(half + 1) * 512],
                start=(I == 0), stop=(I == nI - 1))
        nc.tensor.matmul(dps[:], ones128[:], fold[:],
                         start=(Ip == 0), stop=(Ip == nIp - 1))
        if Ip == nIp - 1:
            rb = attp.tile([128, 512], f32, tag="rbc", bufs=2,
                           name=f"rb_{h}_{J}")
            nc.vector.reciprocal_approx_fast(out=rb[:], in_=dps[:])
            nc.vector.tensor_tensor(
                y_sb[:, h * T + J * 512:h * T + (J + 1) * 512],
                yps[:], rb[:], op=ALU.mult)
            del state[(h, J)]

    LAG = 2
    for t in tasks:
        inflight.append(issue(t))
        if len(inflight) > LAG:
            consume(inflight.pop(0))
    while inflight:
        consume(inflight.pop(0))


def _host_prep(inputs):
    bf = ml_dtypes.bfloat16
    x = np.asarray(inputs["x"], dtype=np.float32)
    Wq = np.asarray(inputs["Wq"], dtype=np.float32)
    Wk = np.asarray(inputs["Wk"], dtype=np.float32)
    Wv = np.asarray(inputs["Wv"], dtype=np.float32)
    Wo = np.asarray(inputs["Wo"], dtype=np.float32)
    w_omega = np.asarray(inputs["w_omega"], dtype=np.float32)
    b_omega = np.asarray(inputs["b_omega"], dtype=np.float32)
    log_freq = np.asarray(inputs["log_freq"], dtype=np.float32)
    q_gamma = np.asarray(inputs["q_gamma"], dtype=np.float32)
    k_gamma = np.asarray(inputs["k_gamma"], dtype=np.float32)

    womg = w_omega.reshape(NCT, 128).T.astype(np.float32)
    # replicated across output rows: womg2[:, i*128+c] = w_omega[i*128+:] col c
    womg2 = np.repeat(womg.T[:, :, None], 128, axis=2)  # [i, 128k, 128c]
    womg2 = womg2.transpose(1, 0, 2).reshape(128, NCT * 128).astype(bf)
    b16 = (b_omega / 16.0).reshape(1, 1).astype(np.float32)
    f = np.exp(log_freq)
    freqs = np.concatenate([f, -f]).reshape(128, 1).astype(np.float32)
    gqv = q_gamma.reshape(128, 1).astype(np.float32)
    gkv = k_gamma.reshape(128, 1).astype(np.float32)
    kk = np.arange(128)
    trilA = (kk[:, None] <= kk[None, :]).astype(bf)  # [k, p] = (k <= p)
    p = np.arange(128)[:, None]
    c = np.arange(512)[None, :]
    maskB = np.concatenate(
        [(NEG * ((p + r * 128) > c)).astype(np.float32) for r in range(4)],
        axis=1).astype(bf)
    ones128 = np.ones((128, 128), dtype=bf)

    in_maps = []
    for core in range(8):
        b, g = core // 2, core % 2
        in_maps.append({
            "xt": np.ascontiguousarray(x[b].T).astype(bf),
            "wq": np.ascontiguousarray(Wq[g * GD:(g + 1) * GD, :].T).astype(bf),
            "wk": np.ascontiguousarray(Wk[g * GD:(g + 1) * GD, :].T).astype(bf),
            "wv": np.ascontiguousarray(Wv[g * GD:(g + 1) * GD, :].T).astype(bf),
            "wo": np.ascontiguousarray(Wo[:, g * GD:(g + 1) * GD].T).astype(bf),
            "womg2": womg2, "b16": b16,
            "freqs": freqs,
            "gq": gqv, "gk": gkv,
            "trilA": trilA, "maskB": maskB, "ones128": ones128,
        })
    return in_maps


def kernel(**inputs) -> np.ndarray:
    if "nc" not in _CACHE:
        _CACHE["nc"] = _build()
    nc = _CACHE["nc"]
    in_maps = _host_prep(inputs)
    res = run_bass_kernel_spmd(nc, in_maps, core_ids=list(range(8)))
    out = np.empty((B, T, C), dtype=np.float32)
    for b in range(B):
        out[b] = res.results[2 * b]["out"] + res.results[2 * b + 1]["out"]
    return out


# revision 16
# speedup vs baseline: 1.0881x; 1.0684x over previous
"""Trainium2 Bass kernel for causal self-attention with cumulative-phase rotary
embedding (nn_CausalSelfAttention_64338610094602).

Sharding: 8 cores = 4 batches x 2 head-groups (tensor-parallel over heads).
Each core computes, for its (batch, 8-head group):
  omega/phi (replicated per batch), QKV projections, rotation + RMSNorm,
  causal attention (transposed-scores layout, max-free softmax), and a
  partial output projection. Host sums the two head-group partials per batch.

v5 design notes (vs v4's per-pair phases):
  - All projections first (P1 omega/trig, P2 all 4 pairs' q/k/v), then one
    flat attention pipeline over all 32 (head, J) block-rows, then P4.
    The PE instruction stream never alternates sections, which avoids both
    the per-row ACT-latency bubbles and the p-state ramp (PE runs at 1.2GHz
    for 3us after any idle gap, 2.4GHz only when continuously busy).
  - q/k (all 8 heads, post-norm, bf16) spill to DRAM during P2 and stream
    back per-head in P3 (SBUF cannot hold 8 heads of q+k next to xts);
    v and y stay SBUF-resident for all heads (no y round-trip).
  - Softmax denominator: each ex2 [128,1024] tile is folded to [128,512]
    on DVE (bf16 add of the two key-tile halves) and the PE ones-matmul
    runs on the folded tile -- half the PE columns of v4's dps.
  - Rotation sign baked into the frequency vector (rows 64:128 negative)
    so trig tiles are written straight out of ACT Sin; gamma applied in the
    RMSNorm multiply (scalar_tensor_tensor) instead of folded into trig.
  - Causal mask folded into the PE score accumulation (trilA x maskB adds
    -1e9*count on diagonal tiles) as in v4.
  - All 4 Wo column blocks prefetched into SBUF during P3; P4 reads y_sb
    directly, so the P3->P4 transition has no DMA wait.
"""
import math

import numpy as np
import ml_dtypes

import concourse.mybir as mybir
import concourse.tile as tile
from concourse import bacc
from concourse.bass_utils import run_bass_kernel_spmd

B, T, C = 4, 2048, 2048
H, D, DH = 16, 128, 64
HG = 8          # heads per core (head-group)
GD = HG * D     # group output dims = 1024
NT = T // 512   # 4 query blocks of 512
NCT = C // 128  # 16 contraction tiles
EPS = 1e-5
SCL = 1.0 / math.sqrt(D)
NEG = -1.0e9

dt = mybir.dt
AF = mybir.ActivationFunctionType
ALU = mybir.AluOpType

TWO_PI = 6.283185307179586
INV_2PI = 1.0 / TWO_PI
CW1 = float(np.float32(6.28125))
CW2 = float(np.float32(TWO_PI - 6.28125))
CW3 = float(TWO_PI - CW1 - float(np.float32(TWO_PI - 6.28125)))
MAGIC = 12582912.0  # 1.5 * 2^23: fp32 add/sub rounds to nearest int
HALF_PI = 1.5707963267948966
PI = 3.141592653589793

_CACHE = {}


def _build():
    f32, bf16 = dt.float32, dt.bfloat16
    nc = bacc.Bacc(None, target_bir_lowering=False)
    with tile.TileContext(nc) as tc:
        # weight inputs are host-prearranged to the exact SBUF layouts so
        # every DMA moves 4KB-contiguous per-partition runs
        xt_d = nc.dram_tensor("xt", (C, T), bf16, kind="ExternalInput")
        wq_d = nc.dram_tensor("wq", (HG, 128, NCT * 128), bf16,
                              kind="ExternalInput")
        wk_d = nc.dram_tensor("wk", (HG, 128, NCT * 128), bf16,
                              kind="ExternalInput")
        wv_d = nc.dram_tensor("wv", (4, 128, NCT * 256), bf16,
                              kind="ExternalInput")
        wo_d = nc.dram_tensor("wo", (128, 4 * HG * 512), bf16,
                              kind="ExternalInput")
        womg2_d = nc.dram_tensor("womg2", (128, NCT * 128), bf16,
                                 kind="ExternalInput")
        b16_d = nc.dram_tensor("b16", (1, 1), f32, kind="ExternalInput")
        freqs_d = nc.dram_tensor("freqs", (128, 1), f32, kind="ExternalInput")
        gq_d = nc.dram_tensor("gq", (128, 1), f32, kind="ExternalInput")
        gk_d = nc.dram_tensor("gk", (128, 1), f32, kind="ExternalInput")
        trilA_d = nc.dram_tensor("trilA", (128, 128), bf16, kind="ExternalInput")
        maskB_d = nc.dram_tensor("maskB", (128, 4 * 512), bf16, kind="ExternalInput")
        ones128_d = nc.dram_tensor("ones128", (128, 128), bf16,
                                   kind="ExternalInput")
        out_d = nc.dram_tensor("out", (T, C), f32, kind="ExternalOutput")

        with tc.tile_pool(name="const", bufs=1) as constp, \
             tc.tile_pool(name="dram", bufs=1, space="DRAM") as dramp, \
             tc.tile_pool(name="core", bufs=1) as corep, \
             tc.tile_pool(name="psp", bufs=1, space="PSUM") as psp:

            # ---- constants ----
            b16t = constp.tile([1, 1], f32)
            nc.sync.dma_start(b16t[:], b16_d[:])
            freqs = constp.tile([128, 1], f32)
            nc.sync.dma_start(freqs[:], freqs_d[:])
            gq = constp.tile([128, 1], f32)
            nc.sync.dma_start(gq[:], gq_d[:])
            gk = constp.tile([128, 1], f32)
            nc.sync.dma_start(gk[:], gk_d[:])
            trilA = constp.tile([128, 128], bf16)
            nc.sync.dma_start(trilA[:], trilA_d[:])
            maskB = constp.tile([128, 4 * 512], bf16)
            nc.sync.dma_start(maskB[:], maskB_d[:])
            ones128 = constp.tile([128, 128], bf16)
            nc.sync.dma_start(ones128[:], ones128_d[:])
            eps128 = constp.tile([128, 1], f32)
            nc.vector.memset(eps128[:], EPS)

            # all-heads v and y stay resident; q/k spill to DRAM (separate
            # tiles so a head's readback only waits on its own spill)
            v_sb = corep.tile([128, 4 * 16 * 256], bf16)  # (pair*16+tt)*256
            y_sb = corep.tile([128, HG * T], bf16)        # yT per head at h*T
            qk_d = {(wi, h): dramp.tile([128, T], bf16, name=f"qkd_{wi}_{h}")
                    for wi in range(2) for h in range(HG)}

            with tc.tile_pool(name="xtp", bufs=1) as xtp, \
                 tc.tile_pool(name="wstp", bufs=1) as wstp, \
                 tc.tile_pool(name="trigp", bufs=1) as trigp:
                trigA = trigp.tile([128, T], bf16)
                trigB = trigp.tile([128, T], bf16)
                _proj(nc, tc, xt_d, wq_d, wk_d, wv_d, womg2_d,
                      xtp, wstp, psp,
                      b16t, freqs, gq, gk, ones128, eps128,
                      trigA, trigB, v_sb, qk_d)

            with tc.tile_pool(name="qkp", bufs=1) as qkp, \
                 tc.tile_pool(name="attp", bufs=1) as attp, \
                 tc.tile_pool(name="p4w", bufs=1) as p4w, \
                 tc.tile_pool(name="p4o", bufs=1) as p4o:
                # stream q/k per head (ring 2), prefetch all wo blocks
                qh_slots = [None, None]

                def fetch_head(h):
                    qh = qkp.tile([128, T], bf16, tag="qh", bufs=2,
                                  name=f"qh_{h}")
                    kh = qkp.tile([128, T], bf16, tag="kh", bufs=2,
                                  name=f"kh_{h}")
                    nc.sync.dma_start(qh[:], qk_d[(0, h)][:])
                    nc.sync.dma_start(kh[:], qk_d[(1, h)][:])
                    qh_slots[h % 2] = (qh, kh)

                fetch_head(0)
                fetch_head(1)
                wo_all = p4w.tile([128, 4 * HG * 512], bf16)  # (cb*8+hh)*512
                for cb in range(4):
                    nc.sync.dma_start(
                        wo_all[:, cb * 4096:(cb + 1) * 4096],
                        wo_d[:, cb * 4096:(cb + 1) * 4096])

                _attention(nc, tc, attp, psp, qh_slots, fetch_head,
                           trilA, maskB, ones128, v_sb, y_sb)

                # ---- P4: out = y^T W_o (partial over heads) ----
                for ti in range(T // 128):
                    for cb in range(4):
                        ops = psp.tile([128, 512], f32, tag="y", bufs=4,
                                       name=f"ops_{ti}_{cb}")
                        for hh in range(HG):
                            nc.tensor.matmul(
                                ops[:],
                                y_sb[:, hh * T + ti * 128:hh * T + (ti + 1) * 128],
                                wo_all[:, (cb * 8 + hh) * 512:(cb * 8 + hh + 1) * 512],
                                start=(hh == 0), stop=(hh == HG - 1))
                        osb = p4o.tile([128, 512], f32, tag="osb", bufs=4)
                        if cb % 2 == 0:
                            nc.scalar.copy(osb[:], ops[:])
                        else:
                            nc.vector.tensor_copy(osb[:], ops[:])
                        nc.sync.dma_start(
                            out_d[ti * 128:(ti + 1) * 128,
                                  cb * 512:(cb + 1) * 512],
                            osb[:])
    nc.compile()
    return nc


def _proj(nc, tc, xt_d, wq_d, wk_d, wv_d, womg2_d,
          xtp, wstp, psp,
          b16t, freqs, gq, gk, ones128, eps128,
          trigA, trigB, v_sb, qk_d):
    f32, bf16 = dt.float32, dt.bfloat16

    sites = [(pair, wi, hl) for pair in range(4) for wi in range(2)
             for hl in range(2)]
    wp_slots = [None, None]
    wvp_slots = [None]

    def issue_panel(si):
        pair, wi, hl = sites[si]
        h = pair * 2 + hl
        w_d = (wq_d, wk_d)[wi]
        wp = wstp.tile([128, NCT * 128], bf16, tag="wp", bufs=2,
                       name=f"wp_{si}")
        nc.sync.dma_start(wp[:], w_d[h, :, :])
        wp_slots[si % 2] = wp

    def issue_wvp(pair):
        wvp = wstp.tile([128, NCT * 256], bf16, tag="wvp", bufs=1,
                        name=f"wvp_{pair}")
        nc.sync.dma_start(wvp[:], wv_d[pair, :, :])
        wvp_slots[0] = wvp

    # ---- P1: omega -> phi -> trig (pools closed before P2's scratch) ----
    # split by T-halves so trig for J0/J1 is ready as soon as the first
    # half of x lands; the x DMA is half-major for the same reason
    with tc.tile_pool(name="p1p", bufs=1) as p1p, \
         tc.tile_pool(name="rowp", bufs=1) as rowp:
        womg2 = p1p.tile([128, NCT * 128], bf16, name="womg2")
        nc.sync.dma_start(womg2[:], womg2_d[:])
        xts = xtp.tile([128, NCT * T], bf16)  # c-tile i at [i*T,(i+1)*T)
        for half in range(2):
            for i in range(NCT):
                cs = half * 1024
                nc.sync.dma_start(
                    xts[:, i * T + cs:i * T + cs + 1024],
                    xt_d[i * 128:(i + 1) * 128, cs:cs + 1024])
            if half == 0:
                issue_panel(0)
        issue_wvp(0)

        HT = T // 2
        omega = rowp.tile([1, T], f32, tag="om")
        incl = rowp.tile([1, T], f32, tag="incl")
        off = rowp.tile([1, 1], f32, tag="off")

        def trig_J(J):
            sl = slice(J * 512, (J + 1) * 512)
            phi2 = p1p.tile([128, 512], f32, tag="p1", bufs=4,
                            name=f"phi2_{J}")
            nc.gpsimd.partition_broadcast(phi2[:], incl[:, sl])
            ang = p1p.tile([128, 512], f32, tag="p1", bufs=4, name=f"ang_{J}")
            # rows 64:128 of freqs are negated: sin rows come out negated,
            # cos rows unchanged (even), which is the rotation's sign layout
            nc.vector.tensor_scalar(ang[:], phi2[:], freqs[:], None,
                                    op0=ALU.mult)
            mm = p1p.tile([128, 512], f32, tag="p1", bufs=4, name=f"mm_{J}")
            nc.vector.tensor_scalar(mm[:], ang[:], INV_2PI, MAGIC,
                                    op0=ALU.mult, op1=ALU.add)
            kk = p1p.tile([128, 512], f32, tag="p1", bufs=4, name=f"kk_{J}")
            nc.vector.tensor_scalar_add(kk[:], mm[:], -MAGIC)
            red = p1p.tile([128, 512], f32, tag="p1", bufs=4, name=f"red_{J}")
            nc.vector.cody_waite_cascade(red[:], ang[:], kk[:], CW1, CW2, CW3)
            red2 = p1p.tile([128, 512], f32, tag="p1", bufs=4,
                            name=f"red2_{J}")
            nc.vector.add_range_wrap(red2[:], red[:], HALF_PI, PI, TWO_PI)
            nc.scalar.activation(trigB[:, sl], red[:], AF.Sin)
            nc.scalar.activation(trigA[:, sl], red2[:], AF.Sin)

        for half in range(2):
            hsl = slice(half * HT, (half + 1) * HT)
            for Jh in range(2):
                J = half * 2 + Jh
                omps = psp.tile([128, 512], f32, tag="y", bufs=4,
                                name=f"omps_{J}")
                for i in range(NCT):
                    nc.tensor.matmul(
                        omps[:], womg2[:, i * 128:(i + 1) * 128],
                        xts[:, i * T + J * 512:i * T + J * 512 + 512],
                        start=(i == 0), stop=(i == NCT - 1))
                nc.scalar.activation(omega[:, J * 512:(J + 1) * 512],
                                     omps[0:1, :],
                                     AF.Sigmoid, scale=1.0 / 16.0,
                                     bias=b16t[:])
            # inclusive scan of this half, then phi (in-place) = incl - omega
            nc.vector.tensor_tensor_scan(incl[:, hsl], omega[:, hsl],
                                         omega[:, hsl], 0.0,
                                         ALU.add, ALU.bypass)
            if half == 0:
                nc.vector.tensor_copy(off[:], incl[:, HT - 1:HT])
            else:
                nc.vector.tensor_scalar(incl[:, hsl], incl[:, hsl],
                                        off[:], None, op0=ALU.add)
            nc.vector.tensor_sub(incl[:, hsl], incl[:, hsl], omega[:, hsl])
            trig_J(half * 2)
            trig_J(half * 2 + 1)

    # ---- P2: q/k/v for all pairs; q/k rotated+normed then spilled ----
    pend_norm = [None]
    pend_tail = [None]

    def flush(pend):
        if pend[0] is not None:
            pend[0]()
            pend[0] = None

    with tc.tile_pool(name="scp", bufs=1) as scp:
        for pair in range(4):
            wvp = wvp_slots[0]

            for wi in range(2):
                for hl in range(2):
                    si = pair * 4 + wi * 2 + hl
                    if si + 1 < len(sites):
                        issue_panel(si + 1)
                    wp = wp_slots[si % 2]
                    h = pair * 2 + hl
                    spill_d = qk_d[(wi, h)]
                    g = (gq, gk)[wi]
                    qsite = scp.tile([128, T], bf16, tag="qk", bufs=2,
                                     name=f"qsite_{si}")
                    sqs = []
                    for Jp in range(2):
                        qps2 = psp.tile([128, 1024], f32, tag="s", bufs=2,
                                        name=f"qps2_{si}_{Jp}")
                        for i in range(NCT):
                            for Jh in range(2):
                                J = 2 * Jp + Jh
                                nc.tensor.matmul(
                                    qps2[:, Jh * 512:(Jh + 1) * 512],
                                    wp[:, i * 128:(i + 1) * 128],
                                    xts[:, i * T + J * 512:i * T + J * 512 + 512],
                                    start=(i == 0), stop=(i == NCT - 1))
                        # flush prev site's ssq tail mid-stream so its rnb
                        # is ready before this site's norm
                        if Jp == 1:
                            flush(pend_tail)
                        for Jh in range(2):
                            J = 2 * Jp + Jh
                            qps = qps2[:, Jh * 512:(Jh + 1) * 512]
                            sl = slice(J * 512, (J + 1) * 512)
                            # rotation: cos part straight into qsite, then
                            # += swapped-half sin part (sign baked in trigB)
                            nc.vector.tensor_tensor(qsite[:, sl], qps,
                                                    trigA[:, sl], op=ALU.mult)
                            Bt = scp.tile([128, 512], f32, tag="rb", bufs=2,
                                          name=f"Bt_{si}_{J}")
                            nc.vector.tensor_tensor(
                                Bt[0:DH, :],
                                qps2[DH:128, Jh * 512:(Jh + 1) * 512],
                                trigB[0:DH, sl], op=ALU.mult)
                            nc.vector.tensor_tensor(
                                Bt[DH:128, :],
                                qps2[0:DH, Jh * 512:(Jh + 1) * 512],
                                trigB[DH:128, sl], op=ALU.mult)
                            nc.vector.tensor_add(
                                qsite[:, sl], qsite[:, sl], Bt[:])
                            # sum-of-squares (rotation preserves norms)
                            sq = scp.tile([128, 512], bf16, tag="sq", bufs=6,
                                          name=f"sq_{si}_{J}")
                            nc.scalar.activation(sq[:], qps, AF.Square)
                            sqs.append((J, sq))
                    flush(pend_norm)

                    def tail(sqs=tuple(sqs), si=si, qsite=qsite, g=g,
                             spill_d=spill_d, pend_norm=pend_norm):
                        rnbs = []
                        for J, sq in sqs:
                            ssqps = psp.tile([128, 512], f32, tag="y", bufs=4,
                                             name=f"ssq_{si}_{J}")
                            nc.tensor.matmul(ssqps[:], ones128[:], sq[:],
                                             start=True, stop=True)
                            rnb = scp.tile([128, 512], bf16, tag="rnb",
                                           bufs=4, name=f"rnb_{si}_{J}")
                            nc.scalar.activation(rnb[:], ssqps[:],
                                                 AF.Abs_reciprocal_sqrt,
                                                 scale=1.0 / 128.0,
                                                 bias=eps128[:])
                            rnbs.append((J, rnb))

                        def norm():
                            for J, rnb in rnbs:
                                sl = slice(J * 512, (J + 1) * 512)
                                nc.vector.scalar_tensor_tensor(
                                    qsite[:, sl], qsite[:, sl], g[:], rnb[:],
                                    op0=ALU.mult, op1=ALU.mult)
                            nc.sync.dma_start(spill_d[:], qsite[:])
                        pend_norm[0] = norm
                    pend_tail[0] = tail

            # --- v for both heads of the pair ---
            vbase = pair * 16 * 256
            for tq in range(4):
                vps = []
                for q4 in range(2):
                    vps.append(psp.tile([128, 1024], f32, tag="s", bufs=2,
                                        name=f"vps_{pair}_{tq}_{q4}"))
                for q4 in range(2):
                    for i in range(NCT):
                        for t2 in range(2):
                            t = q4 * 2 + t2
                            tt = tq * 4 + t
                            nc.tensor.matmul(
                                vps[q4][:, t2 * 512:t2 * 512 + 256],
                                xts[:, i * T + tt * 128:i * T + (tt + 1) * 128],
                                wvp[:, i * 256:(i + 1) * 256],
                                start=(i == 0), stop=(i == NCT - 1))
                for t in range(4):
                    tt = tq * 4 + t
                    # ACT copy: DVE is busy with rotations/norms here and a
                    # lagging copy would stall vps PSUM-bank reuse
                    nc.scalar.copy(
                        v_sb[:, vbase + tt * 256:vbase + (tt + 1) * 256],
                        vps[t // 2][:, (t % 2) * 512:(t % 2) * 512 + 256])
                if tq == 0:
                    flush(pend_tail)
                    flush(pend_norm)
            if pair + 1 < 4:
                issue_wvp(pair + 1)
        flush(pend_tail)
        flush(pend_norm)


def _attention(nc, tc, attp, psp, qh_slots, fetch_head,
               trilA, maskB, ones128, v_sb, y_sb):
    """Flat software pipeline over all (h, J) block-rows at Ip granularity.

    Per task (h, J, Ip): scores for key-tile pair Ip into a [128,1024] PSUM
    tile (mask folded in on diagonal tiles), ACT Exp -> ex2 bf16, DVE fold
    of the two halves for the denominator.  Consumption lags 2 tasks: yps
    matmuls per half plus one dps matmul on the folded tile.  Row epilogue
    (reciprocal + y write) runs on DVE.
    """
    f32, bf16 = dt.float32, dt.bfloat16
    tasks = []
    for h in range(HG):
        for J in range(NT):
            for Ip in range(2 * J + 2):
                tasks.append((h, J, Ip))

    state = {}  # (h, J) -> (yps, dps)
    inflight = []

    def issue(t):
        h, J, Ip = t
        if J == 0 and Ip == 0 and 1 <= h < HG - 1:
            # heads 0/1 are prefetched before the pipeline; ring slot h-1
            # frees once all of head h-1's scores have issued
            fetch_head(h + 1)
        qh, kh = qh_slots[h % 2]
        sps2 = psp.tile([128, 1024], f32, tag="s", bufs=2,
                        name=f"sps_{h}_{J}_{Ip}")
        for half in range(2):
            I = 2 * Ip + half
            diag = I >= 4 * J
            osl = sps2[:, half * 512:(half + 1) * 512]
            nc.tensor.matmul(
                osl,
                kh[:, I * 128:(I + 1) * 128],
                qh[:, J * 512:(J + 1) * 512],
                start=True, stop=(not diag))
            if diag:
                r = I - 4 * J
                nc.tensor.matmul(
                    osl, trilA[:], maskB[:, r * 512:(r + 1) * 512],
                    start=False, stop=True)
        ex2 = attp.tile([128, 1024], bf16, tag="ex", bufs=4,
                        name=f"ex_{h}_{J}_{Ip}")
        nc.scalar.activation(ex2[:], sps2[:], AF.Exp, scale=SCL)
        fold = attp.tile([128, 512], bf16, tag="fold", bufs=4,
                         name=f"fold_{h}_{J}_{Ip}")
        nc.vector.tensor_add(fold[:], ex2[:, 0:512], ex2[:, 512:1024])
        return (t, ex2, fold)

    def consume(item):
        t, ex2, fold = item
        h, J, Ip = t
        nI = 4 * J + 4
        nIp = 2 * J + 2
        if Ip == 0:
            yps = psp.tile([128, 512], f32, tag="y", bufs=4,
                           name=f"yps_{h}_{J}")
            dps = psp.tile([128, 512], f32, tag="y", bufs=4,
                           name=f"dps_{h}_{J}")
            state[(h, J)] = (yps, dps)
        yps, dps = state[(h, J)]
        vbase = (h // 2) * 16 * 256
        hoff = (h % 2) * 128
        for half in range(2):
            I = 2 * Ip + half
            nc.tensor.matmul(
                yps[:],
                v_sb[:, vbase + I * 256 + hoff:vbase + I * 256 + hoff + 128],
                ex2[:, half * 512:(half + 1) * 512],
                start=(I == 0), stop=(I == nI - 1))
        nc.tensor.matmul(dps[:], ones128[:], fold[:],
                         start=(Ip == 0), stop=(Ip == nIp - 1))
        if Ip == nIp - 1:
            rb = attp.tile([128, 512], f32, tag="rbc", bufs=2,
                           name=f"rb_{h}_{J}")
            nc.vector.reciprocal_approx_fast(out=rb[:], in_=dps[:])
            nc.vector.tensor_tensor(
                y_sb[:, h * T + J * 512:h * T + (J + 1) * 512],
                yps[:], rb[:], op=ALU.mult)
            del state[(h, J)]

    LAG = 2
    for t in tasks:
        inflight.append(issue(t))
        if len(inflight) > LAG:
            consume(inflight.pop(0))
    while inflight:
        consume(inflight.pop(0))


def _host_prep(inputs):
    bf = ml_dtypes.bfloat16
    x = np.asarray(inputs["x"], dtype=np.float32)
    Wq = np.asarray(inputs["Wq"], dtype=np.float32)
    Wk = np.asarray(inputs["Wk"], dtype=np.float32)
    Wv = np.asarray(inputs["Wv"], dtype=np.float32)
    Wo = np.asarray(inputs["Wo"], dtype=np.float32)
    w_omega = np.asarray(inputs["w_omega"], dtype=np.float32)
    b_omega = np.asarray(inputs["b_omega"], dtype=np.float32)
    log_freq = np.asarray(inputs["log_freq"], dtype=np.float32)
    q_gamma = np.asarray(inputs["q_gamma"], dtype=np.float32)
    k_gamma = np.asarray(inputs["k_gamma"], dtype=np.float32)

    womg = w_omega.reshape(NCT, 128).T.astype(np.float32)
    # replicated across output rows: womg2[:, i*128+c] = w_omega[i*128+:] col c
    womg2 = np.repeat(womg.T[:, :, None], 128, axis=2)  # [i, 128k, 128c]
    womg2 = womg2.transpose(1, 0, 2).reshape(128, NCT * 128).astype(bf)
    b16 = (b_omega / 16.0).reshape(1, 1).astype(np.float32)
    f = np.exp(log_freq)
    freqs = np.concatenate([f, -f]).reshape(128, 1).astype(np.float32)
    gqv = q_gamma.reshape(128, 1).astype(np.float32)
    gkv = k_gamma.reshape(128, 1).astype(np.float32)
    kk = np.arange(128)
    trilA = (kk[:, None] <= kk[None, :]).astype(bf)  # [k, p] = (k <= p)
    p = np.arange(128)[:, None]
    c = np.arange(512)[None, :]
    maskB = np.concatenate(
        [(NEG * ((p + r * 128) > c)).astype(np.float32) for r in range(4)],
        axis=1).astype(bf)
    ones128 = np.ones((128, 128), dtype=bf)

    def panels_qk(W, g):
        # [h, p, i*128+m] = W_core_T[i*128+p, h*128+m]
        WT = W[g * GD:(g + 1) * GD, :].T  # [C, GD]
        A = WT.reshape(NCT, 128, HG, 128).transpose(2, 1, 0, 3)
        return np.ascontiguousarray(A.reshape(HG, 128, NCT * 128)).astype(bf)

    def panels_v(W, g):
        # [pair, p, i*256+n] = W_core_T[i*128+p, pair*256+n]
        WT = W[g * GD:(g + 1) * GD, :].T
        A = WT.reshape(NCT, 128, 4, 256).transpose(2, 1, 0, 3)
        return np.ascontiguousarray(A.reshape(4, 128, NCT * 256)).astype(bf)

    def panel_o(W, g):
        # [p, (cb*8+hh)*512+c] = W_core_T[hh*128+p, cb*512+c]
        WT = W[:, g * GD:(g + 1) * GD].T  # [GD, C]
        A = WT.reshape(HG, 128, 4, 512).transpose(1, 2, 0, 3)
        return np.ascontiguousarray(A.reshape(128, 4 * HG * 512)).astype(bf)

    in_maps = []
    for core in range(8):
        b, g = core // 2, core % 2
        in_maps.append({
            "xt": np.ascontiguousarray(x[b].T).astype(bf),
            "wq": panels_qk(Wq, g),
            "wk": panels_qk(Wk, g),
            "wv": panels_v(Wv, g),
            "wo": panel_o(Wo, g),
            "womg2": womg2, "b16": b16,
            "freqs": freqs,
            "gq": gqv, "gk": gkv,
            "trilA": trilA, "maskB": maskB, "ones128": ones128,
        })
    return in_maps


def kernel(**inputs) -> np.ndarray:
    if "nc" not in _CACHE:
        _CACHE["nc"] = _build()
    nc = _CACHE["nc"]
    in_maps = _host_prep(inputs)
    res = run_bass_kernel_spmd(nc, in_maps, core_ids=list(range(8)))
    out = np.empty((B, T, C), dtype=np.float32)
    for b in range(B):
        out[b] = res.results[2 * b]["out"] + res.results[2 * b + 1]["out"]
    return out


# revision 21
# speedup vs baseline: 1.1170x; 1.0265x over previous
"""Trainium2 Bass kernel for causal self-attention with cumulative-phase rotary
embedding (nn_CausalSelfAttention_64338610094602).

Sharding: 8 cores = 4 batches x 2 head-groups (tensor-parallel over heads).
Each core computes, for its (batch, 8-head group):
  omega/phi (replicated per batch), QKV projections, rotation + RMSNorm,
  causal attention (transposed-scores layout, max-free softmax), and a
  partial output projection. Host sums the two head-group partials per batch.

v5 design notes (vs v4's per-pair phases):
  - All projections first (P1 omega/trig, P2 all 4 pairs' q/k/v), then one
    flat attention pipeline over all 32 (head, J) block-rows, then P4.
    The PE instruction stream never alternates sections, which avoids both
    the per-row ACT-latency bubbles and the p-state ramp (PE runs at 1.2GHz
    for 3us after any idle gap, 2.4GHz only when continuously busy).
  - q/k (all 8 heads, post-norm, bf16) spill to DRAM during P2 and stream
    back per-head in P3 (SBUF cannot hold 8 heads of q+k next to xts);
    v and y stay SBUF-resident for all heads (no y round-trip).
  - Softmax denominator: each ex2 [128,1024] tile is folded to [128,512]
    on DVE (bf16 add of the two key-tile halves) and the PE ones-matmul
    runs on the folded tile -- half the PE columns of v4's dps.
  - Rotation sign baked into the frequency vector (rows 64:128 negative)
    so trig tiles are written straight out of ACT Sin; gamma applied in the
    RMSNorm multiply (scalar_tensor_tensor) instead of folded into trig.
  - Causal mask folded into the PE score accumulation (trilA x maskB adds
    -1e9*count on diagonal tiles) as in v4.
  - All 4 Wo column blocks prefetched into SBUF during P3; P4 reads y_sb
    directly, so the P3->P4 transition has no DMA wait.
"""
import math

import numpy as np
import ml_dtypes

import concourse.mybir as mybir
import concourse.tile as tile
from concourse import bacc
from concourse.bass_utils import run_bass_kernel_spmd

B, T, C = 4, 2048, 2048
H, D, DH = 16, 128, 64
HG = 8          # heads per core (head-group)
GD = HG * D     # group output dims = 1024
NT = T // 512   # 4 query blocks of 512
NCT = C // 128  # 16 contraction tiles
EPS = 1e-5
SCL = 1.0 / math.sqrt(D)
NEG = -1.0e9

dt = mybir.dt
AF = mybir.ActivationFunctionType
ALU = mybir.AluOpType

TWO_PI = 6.283185307179586
INV_2PI = 1.0 / TWO_PI
CW1 = float(np.float32(6.28125))
CW2 = float(np.float32(TWO_PI - 6.28125))
CW3 = float(TWO_PI - CW1 - float(np.float32(TWO_PI - 6.28125)))
MAGIC = 12582912.0  # 1.5 * 2^23: fp32 add/sub rounds to nearest int
HALF_PI = 1.5707963267948966
PI = 3.141592653589793

_CACHE = {}


def _build():
    f32, bf16 = dt.float32, dt.bfloat16
    nc = bacc.Bacc(None, target_bir_lowering=False)
    with tile.TileContext(nc) as tc:
        # weight inputs are host-prearranged to the exact SBUF layouts so
        # every DMA moves 4KB-contiguous per-partition runs
        xt_d = nc.dram_tensor("xt", (C, T), bf16, kind="ExternalInput")
        wq_d = nc.dram_tensor("wq", (HG, 128, NCT * 128), bf16,
                              kind="ExternalInput")
        wk_d = nc.dram_tensor("wk", (HG, 128, NCT * 128), bf16,
                              kind="ExternalInput")
        wv_d = nc.dram_tensor("wv", (4, 128, NCT * 256), bf16,
                              kind="ExternalInput")
        wo_d = nc.dram_tensor("wo", (128, 4 * HG * 512), bf16,
                              kind="ExternalInput")
        womg2_d = nc.dram_tensor("womg2", (128, NCT * 128), bf16,
                                 kind="ExternalInput")
        b16_d = nc.dram_tensor("b16", (1, 1), f32, kind="ExternalInput")
        freqs_d = nc.dram_tensor("freqs", (128, 1), f32, kind="ExternalInput")
        gq_d = nc.dram_tensor("gq", (128, 1), f32, kind="ExternalInput")
        gk_d = nc.dram_tensor("gk", (128, 1), f32, kind="ExternalInput")
        trilA_d = nc.dram_tensor("trilA", (128, 128), bf16, kind="ExternalInput")
        maskB_d = nc.dram_tensor("maskB", (128, 4 * 512), bf16, kind="ExternalInput")
        ones128_d = nc.dram_tensor("ones128", (128, 128), bf16,
                                   kind="ExternalInput")
        out_d = nc.dram_tensor("out", (T, C), f32, kind="ExternalOutput")

        with tc.tile_pool(name="const", bufs=1) as constp, \
             tc.tile_pool(name="dram", bufs=1, space="DRAM") as dramp, \
             tc.tile_pool(name="core", bufs=1) as corep, \
             tc.tile_pool(name="psp", bufs=1, space="PSUM") as psp:

            # ---- constants ----
            b16t = constp.tile([1, 1], f32)
            nc.sync.dma_start(b16t[:], b16_d[:])
            freqs = constp.tile([128, 1], f32)
            nc.sync.dma_start(freqs[:], freqs_d[:])
            gq = constp.tile([128, 1], f32)
            nc.sync.dma_start(gq[:], gq_d[:])
            gk = constp.tile([128, 1], f32)
            nc.sync.dma_start(gk[:], gk_d[:])
            trilA = constp.tile([128, 128], bf16)
            nc.sync.dma_start(trilA[:], trilA_d[:])
            maskB = constp.tile([128, 4 * 512], bf16)
            nc.sync.dma_start(maskB[:], maskB_d[:])
            ones128 = constp.tile([128, 128], bf16)
            nc.sync.dma_start(ones128[:], ones128_d[:])
            eps128 = constp.tile([128, 1], f32)
            nc.vector.memset(eps128[:], EPS)

            # all-heads v and y stay resident; q/k spill to DRAM (separate
            # tiles so a head's readback only waits on its own spill)
            v_sb = corep.tile([128, 4 * 16 * 256], bf16)  # (pair*16+tt)*256
            y_sb = corep.tile([128, HG * T], bf16)        # yT per head at h*T
            qk_d = {(wi, h): dramp.tile([128, T], bf16, name=f"qkd_{wi}_{h}")
                    for wi in range(2) for h in range(HG)}

            with tc.tile_pool(name="xtp", bufs=1) as xtp, \
                 tc.tile_pool(name="wstp", bufs=1) as wstp, \
                 tc.tile_pool(name="trigp", bufs=1) as trigp:
                trigA = trigp.tile([128, T], bf16)
                trigB = trigp.tile([128, T], bf16)
                _proj(nc, tc, xt_d, wq_d, wk_d, wv_d, womg2_d,
                      xtp, wstp, psp,
                      b16t, freqs, gq, gk, ones128, eps128,
                      trigA, trigB, v_sb, qk_d)

            with tc.tile_pool(name="qkp", bufs=1) as qkp, \
                 tc.tile_pool(name="attp", bufs=1) as attp, \
                 tc.tile_pool(name="p4w", bufs=1) as p4w, \
                 tc.tile_pool(name="p4o", bufs=1) as p4o:
                # stream q/k per head (ring 2), prefetch all wo blocks
                qh_slots = [None, None]

                def fetch_head(h):
                    qh = qkp.tile([128, T], bf16, tag="qh", bufs=2,
                                  name=f"qh_{h}")
                    kh = qkp.tile([128, T], bf16, tag="kh", bufs=2,
                                  name=f"kh_{h}")
                    nc.sync.dma_start(qh[:], qk_d[(0, h)][:])
                    nc.sync.dma_start(kh[:], qk_d[(1, h)][:])
                    qh_slots[h % 2] = (qh, kh)

                fetch_head(0)
                fetch_head(1)
                wo_all = p4w.tile([128, 4 * HG * 512], bf16)  # (cb*8+hh)*512
                for cb in range(4):
                    nc.sync.dma_start(
                        wo_all[:, cb * 4096:(cb + 1) * 4096],
                        wo_d[:, cb * 4096:(cb + 1) * 4096])

                _attention(nc, tc, attp, psp, qh_slots, fetch_head,
                           trilA, maskB, ones128, v_sb, y_sb)

                # ---- P4: out = y^T W_o (partial over heads) ----
                for ti in range(T // 128):
                    for cb in range(4):
                        ops = psp.tile([128, 512], f32, tag="y", bufs=4,
                                       name=f"ops_{ti}_{cb}")
                        for hh in range(HG):
                            nc.tensor.matmul(
                                ops[:],
                                y_sb[:, hh * T + ti * 128:hh * T + (ti + 1) * 128],
                                wo_all[:, (cb * 8 + hh) * 512:(cb * 8 + hh + 1) * 512],
                                start=(hh == 0), stop=(hh == HG - 1))
                        osb = p4o.tile([128, 512], f32, tag="osb", bufs=4)
                        if cb % 2 == 0:
                            nc.scalar.copy(osb[:], ops[:])
                        else:
                            nc.vector.tensor_copy(osb[:], ops[:])
                        nc.sync.dma_start(
                            out_d[ti * 128:(ti + 1) * 128,
                                  cb * 512:(cb + 1) * 512],
                            osb[:])
    nc.compile()
    return nc


def _proj(nc, tc, xt_d, wq_d, wk_d, wv_d, womg2_d,
          xtp, wstp, psp,
          b16t, freqs, gq, gk, ones128, eps128,
          trigA, trigB, v_sb, qk_d):
    f32, bf16 = dt.float32, dt.bfloat16

    sites = [(pair, wi, hl) for pair in range(4) for wi in range(2)
             for hl in range(2)]
    wp_slots = [None, None]
    wvp_slots = [None]

    def issue_panel(si):
        pair, wi, hl = sites[si]
        h = pair * 2 + hl
        w_d = (wq_d, wk_d)[wi]
        wp = wstp.tile([128, NCT * 128], bf16, tag="wp", bufs=2,
                       name=f"wp_{si}")
        nc.sync.dma_start(wp[:], w_d[h, :, :])
        wp_slots[si % 2] = wp

    def issue_wvp(pair):
        wvp = wstp.tile([128, NCT * 256], bf16, tag="wvp", bufs=1,
                        name=f"wvp_{pair}")
        nc.sync.dma_start(wvp[:], wv_d[pair, :, :])
        wvp_slots[0] = wvp

    # ---- P1: omega -> phi -> trig (pools closed before P2's scratch) ----
    # split by T-halves so trig for J0/J1 is ready as soon as the first
    # half of x lands; the x DMA is half-major for the same reason
    with tc.tile_pool(name="p1p", bufs=1) as p1p, \
         tc.tile_pool(name="rowp", bufs=1) as rowp:
        womg2 = p1p.tile([128, NCT * 128], bf16, name="womg2")
        nc.sync.dma_start(womg2[:], womg2_d[:])
        xts = xtp.tile([128, NCT * T], bf16)  # c-tile i at [i*T,(i+1)*T)
        for half in range(2):
            for i in range(NCT):
                cs = half * 1024
                nc.sync.dma_start(
                    xts[:, i * T + cs:i * T + cs + 1024],
                    xt_d[i * 128:(i + 1) * 128, cs:cs + 1024])
            if half == 0:
                issue_panel(0)
        issue_wvp(0)

        HT = T // 2
        omega = rowp.tile([1, T], f32, tag="om")
        incl = rowp.tile([1, T], f32, tag="incl")
        off = rowp.tile([1, 1], f32, tag="off")

        def trig_J(J):
            sl = slice(J * 512, (J + 1) * 512)
            phi2 = p1p.tile([128, 512], f32, tag="p1", bufs=4,
                            name=f"phi2_{J}")
            nc.gpsimd.partition_broadcast(phi2[:], incl[:, sl])
            ang = p1p.tile([128, 512], f32, tag="p1", bufs=4, name=f"ang_{J}")
            # rows 64:128 of freqs are negated: sin rows come out negated,
            # cos rows unchanged (even), which is the rotation's sign layout
            nc.vector.tensor_scalar(ang[:], phi2[:], freqs[:], None,
                                    op0=ALU.mult)
            mm = p1p.tile([128, 512], f32, tag="p1", bufs=4, name=f"mm_{J}")
            nc.vector.tensor_scalar(mm[:], ang[:], INV_2PI, MAGIC,
                                    op0=ALU.mult, op1=ALU.add)
            kk = p1p.tile([128, 512], f32, tag="p1", bufs=4, name=f"kk_{J}")
            nc.vector.tensor_scalar_add(kk[:], mm[:], -MAGIC)
            red = p1p.tile([128, 512], f32, tag="p1", bufs=4, name=f"red_{J}")
            nc.vector.cody_waite_cascade(red[:], ang[:], kk[:], CW1, CW2, CW3)
            red2 = p1p.tile([128, 512], f32, tag="p1", bufs=4,
                            name=f"red2_{J}")
            nc.vector.add_range_wrap(red2[:], red[:], HALF_PI, PI, TWO_PI)
            nc.scalar.activation(trigB[:, sl], red[:], AF.Sin)
            nc.scalar.activation(trigA[:, sl], red2[:], AF.Sin)

        for half in range(2):
            hsl = slice(half * HT, (half + 1) * HT)
            for Jh in range(2):
                J = half * 2 + Jh
                omps = psp.tile([128, 512], f32, tag="y", bufs=4,
                                name=f"omps_{J}")
                for i in range(NCT):
                    nc.tensor.matmul(
                        omps[:], womg2[:, i * 128:(i + 1) * 128],
                        xts[:, i * T + J * 512:i * T + J * 512 + 512],
                        start=(i == 0), stop=(i == NCT - 1))
                nc.scalar.activation(omega[:, J * 512:(J + 1) * 512],
                                     omps[0:1, :],
                                     AF.Sigmoid, scale=1.0 / 16.0,
                                     bias=b16t[:])
            # inclusive scan of this half, then phi (in-place) = incl - omega
            nc.vector.tensor_tensor_scan(incl[:, hsl], omega[:, hsl],
                                         omega[:, hsl], 0.0,
                                         ALU.add, ALU.bypass)
            if half == 0:
                nc.vector.tensor_copy(off[:], incl[:, HT - 1:HT])
            else:
                nc.vector.tensor_scalar(incl[:, hsl], incl[:, hsl],
                                        off[:], None, op0=ALU.add)
            nc.vector.tensor_sub(incl[:, hsl], incl[:, hsl], omega[:, hsl])
            trig_J(half * 2)
            trig_J(half * 2 + 1)

    # ---- P2: q/k/v for all pairs; q/k rotated+normed then spilled ----
    pend_norm = [None]
    pend_tail = [None]

    def flush(pend):
        if pend[0] is not None:
            pend[0]()
            pend[0] = None

    with tc.tile_pool(name="scp", bufs=1) as scp:
        for pair in range(4):
            wvp = wvp_slots[0]

            # --- v first: needs no trig, so the P1 sigmoid->scan->trig
            # chain has cover before the first rotation consumer ---
            vbase = pair * 16 * 256
            for tq in range(4):
                vps = []
                for q4 in range(2):
                    vps.append(psp.tile([128, 1024], f32, tag="s", bufs=2,
                                        name=f"vps_{pair}_{tq}_{q4}"))
                for q4 in range(2):
                    for i in range(NCT):
                        for t2 in range(2):
                            t = q4 * 2 + t2
                            tt = tq * 4 + t
                            nc.tensor.matmul(
                                vps[q4][:, t2 * 512:t2 * 512 + 256],
                                xts[:, i * T + tt * 128:i * T + (tt + 1) * 128],
                                wvp[:, i * 256:(i + 1) * 256],
                                start=(i == 0), stop=(i == NCT - 1))
                for t in range(4):
                    tt = tq * 4 + t
                    # split copies ACT/DVE so neither engine's backlog
                    # stalls vps PSUM-bank reuse
                    dst = v_sb[:, vbase + tt * 256:vbase + (tt + 1) * 256]
                    src = vps[t // 2][:, (t % 2) * 512:(t % 2) * 512 + 256]
                    if t % 2 == 0:
                        nc.scalar.copy(dst, src)
                    else:
                        nc.vector.tensor_copy(dst, src)
                if tq == 0:
                    flush(pend_tail)
                    flush(pend_norm)
            if pair + 1 < 4:
                issue_wvp(pair + 1)

            for wi in range(2):
                for hl in range(2):
                    si = pair * 4 + wi * 2 + hl
                    if si + 1 < len(sites):
                        issue_panel(si + 1)
                    wp = wp_slots[si % 2]
                    h = pair * 2 + hl
                    spill_d = qk_d[(wi, h)]
                    g = (gq, gk)[wi]
                    qsite = scp.tile([128, T], bf16, tag="qk", bufs=2,
                                     name=f"qsite_{si}")
                    sqs = []
                    for Jp in range(2):
                        qps2 = psp.tile([128, 1024], f32, tag="s", bufs=2,
                                        name=f"qps2_{si}_{Jp}")
                        for i in range(NCT):
                            for Jh in range(2):
                                J = 2 * Jp + Jh
                                nc.tensor.matmul(
                                    qps2[:, Jh * 512:(Jh + 1) * 512],
                                    wp[:, i * 128:(i + 1) * 128],
                                    xts[:, i * T + J * 512:i * T + J * 512 + 512],
                                    start=(i == 0), stop=(i == NCT - 1))
                        # flush prev site's ssq tail mid-stream so its rnb
                        # is ready before this site's norm
                        if Jp == 1:
                            flush(pend_tail)
                        for Jh in range(2):
                            J = 2 * Jp + Jh
                            qps = qps2[:, Jh * 512:(Jh + 1) * 512]
                            sl = slice(J * 512, (J + 1) * 512)
                            # rotation: cos part straight into qsite, then
                            # += swapped-half sin part (sign baked in trigB)
                            nc.vector.tensor_tensor(qsite[:, sl], qps,
                                                    trigA[:, sl], op=ALU.mult)
                            Bt = scp.tile([128, 512], f32, tag="rb", bufs=2,
                                          name=f"Bt_{si}_{J}")
                            nc.vector.tensor_tensor(
                                Bt[0:DH, :],
                                qps2[DH:128, Jh * 512:(Jh + 1) * 512],
                                trigB[0:DH, sl], op=ALU.mult)
                            nc.vector.tensor_tensor(
                                Bt[DH:128, :],
                                qps2[0:DH, Jh * 512:(Jh + 1) * 512],
                                trigB[DH:128, sl], op=ALU.mult)
                            nc.vector.tensor_add(
                                qsite[:, sl], qsite[:, sl], Bt[:])
                            # sum-of-squares (rotation preserves norms)
                            sq = scp.tile([128, 512], bf16, tag="sq", bufs=6,
                                          name=f"sq_{si}_{J}")
                            nc.scalar.activation(sq[:], qps, AF.Square)
                            sqs.append((J, sq))
                    flush(pend_norm)

                    def tail(sqs=tuple(sqs), si=si, qsite=qsite, g=g,
                             spill_d=spill_d, pend_norm=pend_norm):
                        rnbs = []
                        for J, sq in sqs:
                            ssqps = psp.tile([128, 512], f32, tag="y", bufs=4,
                                             name=f"ssq_{si}_{J}")
                            nc.tensor.matmul(ssqps[:], ones128[:], sq[:],
                                             start=True, stop=True)
                            rnb = scp.tile([128, 512], bf16, tag="rnb",
                                           bufs=4, name=f"rnb_{si}_{J}")
                            nc.scalar.activation(rnb[:], ssqps[:],
                                                 AF.Abs_reciprocal_sqrt,
                                                 scale=1.0 / 128.0,
                                                 bias=eps128[:])
                            rnbs.append((J, rnb))

                        def norm():
                            for J, rnb in rnbs:
                                sl = slice(J * 512, (J + 1) * 512)
                                nc.vector.scalar_tensor_tensor(
                                    qsite[:, sl], qsite[:, sl], g[:], rnb[:],
                                    op0=ALU.mult, op1=ALU.mult)
                            nc.sync.dma_start(spill_d[:], qsite[:])
                        pend_norm[0] = norm
                    pend_tail[0] = tail

        flush(pend_tail)
        flush(pend_norm)


def _attention(nc, tc, attp, psp, qh_slots, fetch_head,
               trilA, maskB, ones128, v_sb, y_sb):
    """Flat software pipeline over all (h, J) block-rows at Ip granularity.

    Per task (h, J, Ip): scores for key-tile pair Ip into a [128,1024] PSUM
    tile (mask folded in on diagonal tiles), ACT Exp -> ex2 bf16, DVE fold
    of the two halves for the denominator.  Consumption lags 2 tasks: yps
    matmuls per half plus one dps matmul on the folded tile.  Row epilogue
    (reciprocal + y write) runs on DVE.
    """
    f32, bf16 = dt.float32, dt.bfloat16
    tasks = []
    for h in range(HG):
        for J in range(NT):
            for Ip in range(2 * J + 2):
                tasks.append((h, J, Ip))

    state = {}  # (h, J) -> (yps, dps)
    pend_fold = [None]
    inflight = []

    def issue(t):
        h, J, Ip = t
        if J == 0 and Ip == 0 and 1 <= h < HG - 1:
            # heads 0/1 are prefetched before the pipeline; ring slot h-1
            # frees once all of head h-1's scores have issued
            fetch_head(h + 1)
        qh, kh = qh_slots[h % 2]
        sps2 = psp.tile([128, 1024], f32, tag="s", bufs=2,
                        name=f"sps_{h}_{J}_{Ip}")
        for half in range(2):
            I = 2 * Ip + half
            diag = I >= 4 * J
            osl = sps2[:, half * 512:(half + 1) * 512]
            nc.tensor.matmul(
                osl,
                kh[:, I * 128:(I + 1) * 128],
                qh[:, J * 512:(J + 1) * 512],
                start=True, stop=(not diag))
            if diag:
                r = I - 4 * J
                nc.tensor.matmul(
                    osl, trilA[:], maskB[:, r * 512:(r + 1) * 512],
                    start=False, stop=True)
        ex2 = attp.tile([128, 1024], bf16, tag="ex", bufs=4,
                        name=f"ex_{h}_{J}_{Ip}")
        nc.scalar.activation(ex2[:], sps2[:], AF.Exp, scale=SCL)
        fold = attp.tile([128, 512], bf16, tag="fold", bufs=4,
                         name=f"fold_{h}_{J}_{Ip}")
        nc.vector.tensor_add(fold[:], ex2[:, 0:512], ex2[:, 512:1024])
        if Ip % 2 == 0:
            pend_fold[0] = fold
            dps_op = None
        else:
            # second fold level: one dps matmul per 4 key tiles
            dps_op = attp.tile([128, 512], bf16, tag="fold2", bufs=3,
                               name=f"fold2_{h}_{J}_{Ip}")
            nc.vector.tensor_add(dps_op[:], pend_fold[0][:], fold[:])
        return (t, ex2, dps_op)

    def consume(item):
        t, ex2, dps_op = item
        h, J, Ip = t
        nI = 4 * J + 4
        nIp = 2 * J + 2
        if Ip == 0:
            yps = psp.tile([128, 512], f32, tag="y", bufs=4,
                           name=f"yps_{h}_{J}")
            dps = psp.tile([128, 512], f32, tag="y", bufs=4,
                           name=f"dps_{h}_{J}")
            state[(h, J)] = (yps, dps)
        yps, dps = state[(h, J)]
        vbase = (h // 2) * 16 * 256
        hoff = (h % 2) * 128
        for half in range(2):
            I = 2 * Ip + half
            nc.tensor.matmul(
                yps[:],
                v_sb[:, vbase + I * 256 + hoff:vbase + I * 256 + hoff + 128],
                ex2[:, half * 512:(half + 1) * 512],
                start=(I == 0), stop=(I == nI - 1))
        if dps_op is not None:
            nc.tensor.matmul(dps[:], ones128[:], dps_op[:],
                             start=(Ip == 1), stop=(Ip == nIp - 1))
        if Ip == nIp - 1:
            rb = attp.tile([128, 512], f32, tag="rbc", bufs=2,
                           name=f"rb_{h}_{J}")
            nc.vector.reciprocal_approx_fast(out=rb[:], in_=dps[:])
            nc.vector.tensor_tensor(
                y_sb[:, h * T + J * 512:h * T + (J + 1) * 512],
                yps[:], rb[:], op=ALU.mult)
            del state[(h, J)]

    LAG = 2
    for t in tasks:
        inflight.append(issue(t))
        if len(inflight) > LAG:
            consume(inflight.pop(0))
    while inflight:
        consume(inflight.pop(0))


def _host_prep(inputs):
    bf = ml_dtypes.bfloat16
    x = np.asarray(inputs["x"], dtype=np.float32)
    Wq = np.asarray(inputs["Wq"], dtype=np.float32)
    Wk = np.asarray(inputs["Wk"], dtype=np.float32)
    Wv = np.asarray(inputs["Wv"], dtype=np.float32)
    Wo = np.asarray(inputs["Wo"], dtype=np.float32)
    w_omega = np.asarray(inputs["w_omega"], dtype=np.float32)
    b_omega = np.asarray(inputs["b_omega"], dtype=np.float32)
    log_freq = np.asarray(inputs["log_freq"], dtype=np.float32)
    q_gamma = np.asarray(inputs["q_gamma"], dtype=np.float32)
    k_gamma = np.asarray(inputs["k_gamma"], dtype=np.float32)

    womg = w_omega.reshape(NCT, 128).T.astype(np.float32)
    # replicated across output rows: womg2[:, i*128+c] = w_omega[i*128+:] col c
    womg2 = np.repeat(womg.T[:, :, None], 128, axis=2)  # [i, 128k, 128c]
    womg2 = womg2.transpose(1, 0, 2).reshape(128, NCT * 128).astype(bf)
    b16 = (b_omega / 16.0).reshape(1, 1).astype(np.float32)
    f = np.exp(log_freq)
    freqs = np.concatenate([f, -f]).reshape(128, 1).astype(np.float32)
    gqv = q_gamma.reshape(128, 1).astype(np.float32)
    gkv = k_gamma.reshape(128, 1).astype(np.float32)
    kk = np.arange(128)
    trilA = (kk[:, None] <= kk[None, :]).astype(bf)  # [k, p] = (k <= p)
    p = np.arange(128)[:, None]
    c = np.arange(512)[None, :]
    maskB = np.concatenate(
        [(NEG * ((p + r * 128) > c)).astype(np.float32) for r in range(4)],
        axis=1).astype(bf)
    ones128 = np.ones((128, 128), dtype=bf)

    def panels_qk(W, g):
        # [h, p, i*128+m] = W_core_T[i*128+p, h*128+m]
        WT = W[g * GD:(g + 1) * GD, :].T  # [C, GD]
        A = WT.reshape(NCT, 128, HG, 128).transpose(2, 1, 0, 3)
        return np.ascontiguousarray(A.reshape(HG, 128, NCT * 128)).astype(bf)

    def panels_v(W, g):
        # [pair, p, i*256+n] = W_core_T[i*128+p, pair*256+n]
        WT = W[g * GD:(g + 1) * GD, :].T
        A = WT.reshape(NCT, 128, 4, 256).transpose(2, 1, 0, 3)
        return np.ascontiguousarray(A.reshape(4, 128, NCT * 256)).astype(bf)

    def panel_o(W, g):
        # [p, (cb*8+hh)*512+c] = W_core_T[hh*128+p, cb*512+c]
        WT = W[:, g * GD:(g + 1) * GD].T  # [GD, C]
        A = WT.reshape(HG, 128, 4, 512).transpose(1, 2, 0, 3)
        return np.ascontiguousarray(A.reshape(128, 4 * HG * 512)).astype(bf)

    in_maps = []
    for core in range(8):
        b, g = core // 2, core % 2
        in_maps.append({
            "xt": np.ascontiguousarray(x[b].T).astype(bf),
            "wq": panels_qk(Wq, g),
            "wk": panels_qk(Wk, g),
            "wv": panels_v(Wv, g),
            "wo": panel_o(Wo, g),
            "womg2": womg2, "b16": b16,
            "freqs": freqs,
            "gq": gqv, "gk": gkv,
            "trilA": trilA, "maskB": maskB, "ones128": ones128,
        })
    return in_maps


def kernel(**inputs) -> np.ndarray:
    if "nc" not in _CACHE:
        _CACHE["nc"] = _build()
    nc = _CACHE["nc"]
    in_maps = _host_prep(inputs)
    res = run_bass_kernel_spmd(nc, in_maps, core_ids=list(range(8)))
    out = np.empty((B, T, C), dtype=np.float32)
    for b in range(B):
        out[b] = res.results[2 * b]["out"] + res.results[2 * b + 1]["out"]
    return out


# revision 26
# speedup vs baseline: 1.1329x; 1.0142x over previous
"""Trainium2 Bass kernel for causal self-attention with cumulative-phase rotary
embedding (nn_CausalSelfAttention_64338610094602).

Sharding: 8 cores = 4 batches x 2 head-groups (tensor-parallel over heads).
Each core computes, for its (batch, 8-head group):
  omega/phi (replicated per batch), QKV projections, rotation + RMSNorm,
  causal attention (transposed-scores layout, max-free softmax), and a
  partial output projection. Host sums the two head-group partials per batch.

v5 design notes (vs v4's per-pair phases):
  - All projections first (P1 omega/trig, P2 all 4 pairs' q/k/v), then one
    flat attention pipeline over all 32 (head, J) block-rows, then P4.
    The PE instruction stream never alternates sections, which avoids both
    the per-row ACT-latency bubbles and the p-state ramp (PE runs at 1.2GHz
    for 3us after any idle gap, 2.4GHz only when continuously busy).
  - q/k (all 8 heads, post-norm, bf16) spill to DRAM during P2 and stream
    back per-head in P3 (SBUF cannot hold 8 heads of q+k next to xts);
    v and y stay SBUF-resident for all heads (no y round-trip).
  - Softmax denominator: each ex2 [128,1024] tile is folded to [128,512]
    on DVE (bf16 add of the two key-tile halves) and the PE ones-matmul
    runs on the folded tile -- half the PE columns of v4's dps.
  - Rotation sign baked into the frequency vector (rows 64:128 negative)
    so trig tiles are written straight out of ACT Sin; gamma applied in the
    RMSNorm multiply (scalar_tensor_tensor) instead of folded into trig.
  - Causal mask folded into the PE score accumulation (trilA x maskB adds
    -1e9*count on diagonal tiles) as in v4.
  - All 4 Wo column blocks prefetched into SBUF during P3; P4 reads y_sb
    directly, so the P3->P4 transition has no DMA wait.
"""
import math

import numpy as np
import ml_dtypes

import concourse.mybir as mybir
import concourse.tile as tile
from concourse import bacc
from concourse.bass_utils import run_bass_kernel_spmd

B, T, C = 4, 2048, 2048
H, D, DH = 16, 128, 64
HG = 8          # heads per core (head-group)
GD = HG * D     # group output dims = 1024
NT = T // 512   # 4 query blocks of 512
NCT = C // 128  # 16 contraction tiles
EPS = 1e-5
SCL = 1.0 / math.sqrt(D)
NEG = -1.0e9

dt = mybir.dt
AF = mybir.ActivationFunctionType
ALU = mybir.AluOpType

TWO_PI = 6.283185307179586
INV_2PI = 1.0 / TWO_PI
CW1 = float(np.float32(6.28125))
CW2 = float(np.float32(TWO_PI - 6.28125))
CW3 = float(TWO_PI - CW1 - float(np.float32(TWO_PI - 6.28125)))
MAGIC = 12582912.0  # 1.5 * 2^23: fp32 add/sub rounds to nearest int
HALF_PI = 1.5707963267948966
PI = 3.141592653589793

_CACHE = {}


def _build():
    f32, bf16 = dt.float32, dt.bfloat16
    nc = bacc.Bacc(None, target_bir_lowering=False)
    with tile.TileContext(nc) as tc:
        # weight inputs are host-prearranged to the exact SBUF layouts so
        # every DMA moves 4KB-contiguous per-partition runs
        xt_d = nc.dram_tensor("xt", (C, T), bf16, kind="ExternalInput")
        wq_d = nc.dram_tensor("wq", (HG, 128, NCT * 128), bf16,
                              kind="ExternalInput")
        wk_d = nc.dram_tensor("wk", (HG, 128, NCT * 128), bf16,
                              kind="ExternalInput")
        wv_d = nc.dram_tensor("wv", (4, 128, NCT * 256), bf16,
                              kind="ExternalInput")
        wo_d = nc.dram_tensor("wo", (128, 4 * HG * 512), bf16,
                              kind="ExternalInput")
        womg2_d = nc.dram_tensor("womg2", (128, NCT * 128), bf16,
                                 kind="ExternalInput")
        b16_d = nc.dram_tensor("b16", (1, 1), f32, kind="ExternalInput")
        freqs_d = nc.dram_tensor("freqs", (128, 1), f32, kind="ExternalInput")
        gq_d = nc.dram_tensor("gq", (128, 1), f32, kind="ExternalInput")
        gk_d = nc.dram_tensor("gk", (128, 1), f32, kind="ExternalInput")
        maskB_d = nc.dram_tensor("maskB", (128, 4 * 512), bf16, kind="ExternalInput")
        ones128_d = nc.dram_tensor("ones128", (128, 128), bf16,
                                   kind="ExternalInput")
        out_d = nc.dram_tensor("out", (T, C), f32, kind="ExternalOutput")

        with tc.tile_pool(name="const", bufs=1) as constp, \
             tc.tile_pool(name="dram", bufs=1, space="DRAM") as dramp, \
             tc.tile_pool(name="core", bufs=1) as corep, \
             tc.tile_pool(name="psp", bufs=1, space="PSUM") as psp:

            # ---- constants ----
            b16t = constp.tile([1, 1], f32)
            nc.sync.dma_start(b16t[:], b16_d[:])
            freqs = constp.tile([128, 1], f32)
            nc.sync.dma_start(freqs[:], freqs_d[:])
            gq = constp.tile([128, 1], f32)
            nc.sync.dma_start(gq[:], gq_d[:])
            gk = constp.tile([128, 1], f32)
            nc.sync.dma_start(gk[:], gk_d[:])
            maskB = constp.tile([128, 4 * 512], bf16)
            nc.sync.dma_start(maskB[:], maskB_d[:])
            ones128 = constp.tile([128, 128], bf16)
            nc.sync.dma_start(ones128[:], ones128_d[:])
            eps128 = constp.tile([128, 1], f32)
            nc.vector.memset(eps128[:], EPS)

            # all-heads v and y stay resident; q/k spill to DRAM (separate
            # tiles so a head's readback only waits on its own spill)
            v_sb = corep.tile([128, 4 * 16 * 256], bf16)  # (pair*16+tt)*256
            y_sb = corep.tile([128, HG * T], bf16)        # yT per head at h*T
            qk_d = {(wi, h): dramp.tile([128, T], bf16, name=f"qkd_{wi}_{h}")
                    for wi in range(2) for h in range(HG)}

            with tc.tile_pool(name="xtp", bufs=1) as xtp, \
                 tc.tile_pool(name="wstp", bufs=1) as wstp, \
                 tc.tile_pool(name="trigp", bufs=1) as trigp:
                trigA = trigp.tile([128, T], bf16)
                trigB = trigp.tile([128, T], bf16)
                _proj(nc, tc, xt_d, wq_d, wk_d, wv_d, womg2_d,
                      xtp, wstp, psp,
                      b16t, freqs, gq, gk, ones128, eps128,
                      trigA, trigB, v_sb, qk_d)

            with tc.tile_pool(name="qkp", bufs=1) as qkp, \
                 tc.tile_pool(name="attp", bufs=1) as attp, \
                 tc.tile_pool(name="p4w", bufs=1) as p4w, \
                 tc.tile_pool(name="p4o", bufs=1) as p4o:
                # stream q/k per head (ring 2), prefetch all wo blocks
                qh_slots = [None, None]

                def fetch_head(h):
                    qh = qkp.tile([128, T], bf16, tag="qh", bufs=2,
                                  name=f"qh_{h}")
                    kh = qkp.tile([128, T], bf16, tag="kh", bufs=2,
                                  name=f"kh_{h}")
                    for c in range(2):
                        sl = slice(c * 1024, (c + 1) * 1024)
                        nc.sync.dma_start(qh[:, sl], qk_d[(0, h)][:, sl])
                        nc.sync.dma_start(kh[:, sl], qk_d[(1, h)][:, sl])
                    qh_slots[h % 2] = (qh, kh)

                fetch_head(0)
                fetch_head(1)
                wo_all = p4w.tile([128, 4 * HG * 512], bf16)  # (cb*8+hh)*512
                for cb in range(4):
                    for c in range(2):
                        sl = slice(cb * 4096 + c * 2048,
                                   cb * 4096 + (c + 1) * 2048)
                        nc.sync.dma_start(wo_all[:, sl], wo_d[:, sl])

                _attention(nc, tc, attp, psp, qh_slots, fetch_head,
                           maskB, ones128, v_sb, y_sb)

                # ---- P4: out = y^T W_o (partial over heads) ----
                for ti in range(T // 128):
                    for cb in range(4):
                        ops = psp.tile([128, 512], f32, tag="y", bufs=4,
                                       name=f"ops_{ti}_{cb}")
                        for hh in range(HG):
                            nc.tensor.matmul(
                                ops[:],
                                y_sb[:, hh * T + ti * 128:hh * T + (ti + 1) * 128],
                                wo_all[:, (cb * 8 + hh) * 512:(cb * 8 + hh + 1) * 512],
                                start=(hh == 0), stop=(hh == HG - 1))
                        osb = p4o.tile([128, 512], f32, tag="osb", bufs=4)
                        if cb % 2 == 0:
                            nc.scalar.copy(osb[:], ops[:])
                        else:
                            nc.vector.tensor_copy(osb[:], ops[:])
                        nc.sync.dma_start(
                            out_d[ti * 128:(ti + 1) * 128,
                                  cb * 512:(cb + 1) * 512],
                            osb[:])
    nc.compile()
    return nc


def _proj(nc, tc, xt_d, wq_d, wk_d, wv_d, womg2_d,
          xtp, wstp, psp,
          b16t, freqs, gq, gk, ones128, eps128,
          trigA, trigB, v_sb, qk_d):
    f32, bf16 = dt.float32, dt.bfloat16

    sites = [(pair, wi, hl) for pair in range(4) for wi in range(2)
             for hl in range(2)]
    wp_slots = [None, None]
    wvp_slots = [None]

    # each dma_start lands on one ~22GB/s queue: split panel transfers into
    # chunks so they spread across queues (runs stay 4KB-contiguous)
    def issue_panel(si):
        pair, wi, hl = sites[si]
        h = pair * 2 + hl
        w_d = (wq_d, wk_d)[wi]
        wp = wstp.tile([128, NCT * 128], bf16, tag="wp", bufs=2,
                       name=f"wp_{si}")
        for c in range(2):
            nc.sync.dma_start(wp[:, c * 1024:(c + 1) * 1024],
                              w_d[h, :, c * 1024:(c + 1) * 1024])
        wp_slots[si % 2] = wp

    def issue_wvp(pair):
        wvp = wstp.tile([128, NCT * 256], bf16, tag="wvp", bufs=1,
                        name=f"wvp_{pair}")
        for c in range(4):
            nc.sync.dma_start(wvp[:, c * 1024:(c + 1) * 1024],
                              wv_d[pair, :, c * 1024:(c + 1) * 1024])
        wvp_slots[0] = wvp

    # ---- P1: omega -> phi -> trig (pools closed before P2's scratch) ----
    # split by T-halves so trig for J0/J1 is ready as soon as the first
    # half of x lands; the x DMA is half-major for the same reason
    with tc.tile_pool(name="p1p", bufs=1) as p1p, \
         tc.tile_pool(name="rowp", bufs=1) as rowp:
        womg2 = p1p.tile([128, NCT * 128], bf16, name="womg2")
        for c in range(4):
            nc.sync.dma_start(womg2[:, c * 512:(c + 1) * 512],
                              womg2_d[:, c * 512:(c + 1) * 512])
        xts = xtp.tile([128, NCT * T], bf16)  # c-tile i at [i*T,(i+1)*T)
        for half in range(2):
            for i in range(NCT):
                cs = half * 1024
                nc.sync.dma_start(
                    xts[:, i * T + cs:i * T + cs + 1024],
                    xt_d[i * 128:(i + 1) * 128, cs:cs + 1024])
            if half == 0:
                issue_panel(0)
        issue_wvp(0)

        HT = T // 2
        omega = rowp.tile([1, T], f32, tag="om")
        incl = rowp.tile([1, T], f32, tag="incl")
        off = rowp.tile([1, 1], f32, tag="off")

        def trig_J(J):
            sl = slice(J * 512, (J + 1) * 512)
            phi2 = p1p.tile([128, 512], f32, tag="p1", bufs=4,
                            name=f"phi2_{J}")
            nc.gpsimd.partition_broadcast(phi2[:], incl[:, sl])
            ang = p1p.tile([128, 512], f32, tag="p1", bufs=4, name=f"ang_{J}")
            # rows 64:128 of freqs are negated: sin rows come out negated,
            # cos rows unchanged (even), which is the rotation's sign layout
            nc.vector.tensor_scalar(ang[:], phi2[:], freqs[:], None,
                                    op0=ALU.mult)
            mm = p1p.tile([128, 512], f32, tag="p1", bufs=4, name=f"mm_{J}")
            nc.vector.tensor_scalar(mm[:], ang[:], INV_2PI, MAGIC,
                                    op0=ALU.mult, op1=ALU.add)
            kk = p1p.tile([128, 512], f32, tag="p1", bufs=4, name=f"kk_{J}")
            nc.vector.tensor_scalar_add(kk[:], mm[:], -MAGIC)
            red = p1p.tile([128, 512], f32, tag="p1", bufs=4, name=f"red_{J}")
            nc.vector.cody_waite_cascade(red[:], ang[:], kk[:], CW1, CW2, CW3)
            red2 = p1p.tile([128, 512], f32, tag="p1", bufs=4,
                            name=f"red2_{J}")
            nc.vector.add_range_wrap(red2[:], red[:], HALF_PI, PI, TWO_PI)
            nc.scalar.activation(trigB[:, sl], red[:], AF.Sin)
            nc.scalar.activation(trigA[:, sl], red2[:], AF.Sin)

        for half in range(2):
            hsl = slice(half * HT, (half + 1) * HT)
            for Jh in range(2):
                J = half * 2 + Jh
                omps = psp.tile([128, 512], f32, tag="y", bufs=4,
                                name=f"omps_{J}")
                for i in range(NCT):
                    nc.tensor.matmul(
                        omps[:], womg2[:, i * 128:(i + 1) * 128],
                        xts[:, i * T + J * 512:i * T + J * 512 + 512],
                        start=(i == 0), stop=(i == NCT - 1))
                nc.scalar.activation(omega[:, J * 512:(J + 1) * 512],
                                     omps[0:1, :],
                                     AF.Sigmoid, scale=1.0 / 16.0,
                                     bias=b16t[:])
            # inclusive scan of this half, then phi (in-place) = incl - omega
            nc.vector.tensor_tensor_scan(incl[:, hsl], omega[:, hsl],
                                         omega[:, hsl], 0.0,
                                         ALU.add, ALU.bypass)
            if half == 0:
                nc.vector.tensor_copy(off[:], incl[:, HT - 1:HT])
            else:
                nc.vector.tensor_scalar(incl[:, hsl], incl[:, hsl],
                                        off[:], None, op0=ALU.add)
            nc.vector.tensor_sub(incl[:, hsl], incl[:, hsl], omega[:, hsl])
            trig_J(half * 2)
            trig_J(half * 2 + 1)

    # ---- P2: q/k/v for all pairs; q/k rotated+normed then spilled ----
    pend_norm = [None]
    pend_tail = [None]

    def flush(pend):
        if pend[0] is not None:
            pend[0]()
            pend[0] = None

    with tc.tile_pool(name="scp", bufs=1) as scp:
        for pair in range(4):
            wvp = wvp_slots[0]

            # --- v first: needs no trig, so the P1 sigmoid->scan->trig
            # chain has cover before the first rotation consumer ---
            vbase = pair * 16 * 256
            for tq in range(4):
                vps = []
                for q4 in range(2):
                    vps.append(psp.tile([128, 1024], f32, tag="s", bufs=2,
                                        name=f"vps_{pair}_{tq}_{q4}"))
                for q4 in range(2):
                    for i in range(NCT):
                        for t2 in range(2):
                            t = q4 * 2 + t2
                            tt = tq * 4 + t
                            nc.tensor.matmul(
                                vps[q4][:, t2 * 512:t2 * 512 + 256],
                                xts[:, i * T + tt * 128:i * T + (tt + 1) * 128],
                                wvp[:, i * 256:(i + 1) * 256],
                                start=(i == 0), stop=(i == NCT - 1))
                for t in range(4):
                    tt = tq * 4 + t
                    # split copies ACT/DVE so neither engine's backlog
                    # stalls vps PSUM-bank reuse
                    dst = v_sb[:, vbase + tt * 256:vbase + (tt + 1) * 256]
                    src = vps[t // 2][:, (t % 2) * 512:(t % 2) * 512 + 256]
                    if t % 2 == 0:
                        nc.scalar.copy(dst, src)
                    else:
                        nc.vector.tensor_copy(dst, src)
                if tq == 0:
                    flush(pend_tail)
                    flush(pend_norm)
            if pair + 1 < 4:
                issue_wvp(pair + 1)

            for wi in range(2):
                for hl in range(2):
                    si = pair * 4 + wi * 2 + hl
                    if si + 1 < len(sites):
                        issue_panel(si + 1)
                    wp = wp_slots[si % 2]
                    h = pair * 2 + hl
                    spill_d = qk_d[(wi, h)]
                    g = (gq, gk)[wi]
                    qsite = scp.tile([128, T], bf16, tag="qk", bufs=2,
                                     name=f"qsite_{si}")
                    sqs = []
                    for Jp in range(2):
                        qps2 = psp.tile([128, 1024], f32, tag="s", bufs=2,
                                        name=f"qps2_{si}_{Jp}")
                        for i in range(NCT):
                            for Jh in range(2):
                                J = 2 * Jp + Jh
                                nc.tensor.matmul(
                                    qps2[:, Jh * 512:(Jh + 1) * 512],
                                    wp[:, i * 128:(i + 1) * 128],
                                    xts[:, i * T + J * 512:i * T + J * 512 + 512],
                                    start=(i == 0), stop=(i == NCT - 1))
                        # flush prev site's ssq tail mid-stream so its rnb
                        # is ready before this site's norm
                        if Jp == 1:
                            flush(pend_tail)
                        for Jh in range(2):
                            J = 2 * Jp + Jh
                            qps = qps2[:, Jh * 512:(Jh + 1) * 512]
                            sl = slice(J * 512, (J + 1) * 512)
                            # rotation: cos part straight into qsite, then
                            # += swapped-half sin part (sign baked in trigB)
                            nc.vector.tensor_tensor(qsite[:, sl], qps,
                                                    trigA[:, sl], op=ALU.mult)
                            Bt = scp.tile([128, 512], f32, tag="rb", bufs=2,
                                          name=f"Bt_{si}_{J}")
                            nc.vector.tensor_tensor(
                                Bt[0:DH, :],
                                qps2[DH:128, Jh * 512:(Jh + 1) * 512],
                                trigB[0:DH, sl], op=ALU.mult)
                            nc.vector.tensor_tensor(
                                Bt[DH:128, :],
                                qps2[0:DH, Jh * 512:(Jh + 1) * 512],
                                trigB[DH:128, sl], op=ALU.mult)
                            nc.vector.tensor_add(
                                qsite[:, sl], qsite[:, sl], Bt[:])
                            # sum-of-squares (rotation preserves norms)
                            sq = scp.tile([128, 512], bf16, tag="sq", bufs=6,
                                          name=f"sq_{si}_{J}")
                            nc.scalar.activation(sq[:], qps, AF.Square)
                            sqs.append((J, sq))
                    flush(pend_norm)

                    def tail(sqs=tuple(sqs), si=si, qsite=qsite, g=g,
                             spill_d=spill_d, pend_norm=pend_norm):
                        rnbs = []
                        for J, sq in sqs:
                            ssqps = psp.tile([128, 512], f32, tag="y", bufs=4,
                                             name=f"ssq_{si}_{J}")
                            nc.tensor.matmul(ssqps[:], ones128[:], sq[:],
                                             start=True, stop=True)
                            rnb = scp.tile([128, 512], bf16, tag="rnb",
                                           bufs=4, name=f"rnb_{si}_{J}")
                            nc.scalar.activation(rnb[:], ssqps[:],
                                                 AF.Abs_reciprocal_sqrt,
                                                 scale=1.0 / 128.0,
                                                 bias=eps128[:])
                            rnbs.append((J, rnb))

                        def norm():
                            for J, rnb in rnbs:
                                sl = slice(J * 512, (J + 1) * 512)
                                nc.vector.scalar_tensor_tensor(
                                    qsite[:, sl], qsite[:, sl], g[:], rnb[:],
                                    op0=ALU.mult, op1=ALU.mult)
                            nc.sync.dma_start(spill_d[:], qsite[:])
                        pend_norm[0] = norm
                    pend_tail[0] = tail

        flush(pend_tail)
        flush(pend_norm)


def _attention(nc, tc, attp, psp, qh_slots, fetch_head,
               maskB, ones128, v_sb, y_sb):
    """Flat software pipeline over all (h, J) block-rows at Ip granularity.

    Per task (h, J, Ip): scores for key-tile pair Ip into a [128,1024] PSUM
    tile (mask folded in on diagonal tiles), ACT Exp -> ex2 bf16, DVE fold
    of the two halves for the denominator.  Consumption lags 2 tasks: yps
    matmuls per half plus one dps matmul on the folded tile.  Row epilogue
    (reciprocal + y write) runs on DVE.
    """
    f32, bf16 = dt.float32, dt.bfloat16
    tasks = []
    for h in range(HG):
        for J in range(NT):
            for Ip in range(2 * J + 2):
                tasks.append((h, J, Ip))

    state = {}  # (h, J) -> (yps, dps)
    pend_fold = [None]
    inflight = []

    def issue(t):
        h, J, Ip = t
        if J == 0 and Ip == 0 and 1 <= h < HG - 1:
            # heads 0/1 are prefetched before the pipeline; ring slot h-1
            # frees once all of head h-1's scores have issued
            fetch_head(h + 1)
        qh, kh = qh_slots[h % 2]
        sps2 = psp.tile([128, 1024], f32, tag="s", bufs=2,
                        name=f"sps_{h}_{J}_{Ip}")
        for half in range(2):
            I = 2 * Ip + half
            osl = sps2[:, half * 512:(half + 1) * 512]
            nc.tensor.matmul(
                osl,
                kh[:, I * 128:(I + 1) * 128],
                qh[:, J * 512:(J + 1) * 512],
                start=True, stop=True)
        ex2 = attp.tile([128, 1024], bf16, tag="ex", bufs=4,
                        name=f"ex_{h}_{J}_{Ip}")
        nc.scalar.activation(ex2[:], sps2[:], AF.Exp, scale=SCL)
        if 2 * Ip >= 4 * J:
            # causal mask: zero the upper-triangular part of the two
            # diagonal key tiles with one in-place 0/1 multiply (DVE);
            # cheaper than PE mask matmuls and off the critical engine
            r = 2 * Ip - 4 * J
            nc.vector.tensor_tensor(ex2[:], ex2[:],
                                    maskB[:, r * 512:r * 512 + 1024],
                                    op=ALU.mult)
        fold = attp.tile([128, 512], bf16, tag="fold", bufs=4,
                         name=f"fold_{h}_{J}_{Ip}")
        nc.vector.tensor_add(fold[:], ex2[:, 0:512], ex2[:, 512:1024])
        if Ip % 2 == 0:
            pend_fold[0] = fold
            dps_op = None
        else:
            # second fold level: one dps matmul per 4 key tiles
            dps_op = attp.tile([128, 512], bf16, tag="fold2", bufs=3,
                               name=f"fold2_{h}_{J}_{Ip}")
            nc.vector.tensor_add(dps_op[:], pend_fold[0][:], fold[:])
        return (t, ex2, dps_op)

    def consume(item):
        t, ex2, dps_op = item
        h, J, Ip = t
        nI = 4 * J + 4
        nIp = 2 * J + 2
        if Ip == 0:
            yps = psp.tile([128, 512], f32, tag="y", bufs=4,
                           name=f"yps_{h}_{J}")
            dps = psp.tile([128, 512], f32, tag="y", bufs=4,
                           name=f"dps_{h}_{J}")
            state[(h, J)] = (yps, dps)
        yps, dps = state[(h, J)]
        vbase = (h // 2) * 16 * 256
        hoff = (h % 2) * 128
        for half in range(2):
            I = 2 * Ip + half
            nc.tensor.matmul(
                yps[:],
                v_sb[:, vbase + I * 256 + hoff:vbase + I * 256 + hoff + 128],
                ex2[:, half * 512:(half + 1) * 512],
                start=(I == 0), stop=(I == nI - 1))
        if dps_op is not None:
            nc.tensor.matmul(dps[:], ones128[:], dps_op[:],
                             start=(Ip == 1), stop=(Ip == nIp - 1))
        if Ip == nIp - 1:
            rb = attp.tile([128, 512], f32, tag="rbc", bufs=2,
                           name=f"rb_{h}_{J}")
            nc.vector.reciprocal_approx_fast(out=rb[:], in_=dps[:])
            nc.vector.tensor_tensor(
                y_sb[:, h * T + J * 512:h * T + (J + 1) * 512],
                yps[:], rb[:], op=ALU.mult)
            del state[(h, J)]

    LAG = 2
    for t in tasks:
        inflight.append(issue(t))
        if len(inflight) > LAG:
            consume(inflight.pop(0))
    while inflight:
        consume(inflight.pop(0))


def _host_prep(inputs):
    bf = ml_dtypes.bfloat16
    x = np.asarray(inputs["x"], dtype=np.float32)
    Wq = np.asarray(inputs["Wq"], dtype=np.float32)
    Wk = np.asarray(inputs["Wk"], dtype=np.float32)
    Wv = np.asarray(inputs["Wv"], dtype=np.float32)
    Wo = np.asarray(inputs["Wo"], dtype=np.float32)
    w_omega = np.asarray(inputs["w_omega"], dtype=np.float32)
    b_omega = np.asarray(inputs["b_omega"], dtype=np.float32)
    log_freq = np.asarray(inputs["log_freq"], dtype=np.float32)
    q_gamma = np.asarray(inputs["q_gamma"], dtype=np.float32)
    k_gamma = np.asarray(inputs["k_gamma"], dtype=np.float32)

    womg = w_omega.reshape(NCT, 128).T.astype(np.float32)
    # replicated across output rows: womg2[:, i*128+c] = w_omega[i*128+:] col c
    womg2 = np.repeat(womg.T[:, :, None], 128, axis=2)  # [i, 128k, 128c]
    womg2 = womg2.transpose(1, 0, 2).reshape(128, NCT * 128).astype(bf)
    b16 = (b_omega / 16.0).reshape(1, 1).astype(np.float32)
    f = np.exp(log_freq)
    freqs = np.concatenate([f, -f]).reshape(128, 1).astype(np.float32)
    gqv = q_gamma.reshape(128, 1).astype(np.float32)
    gkv = k_gamma.reshape(128, 1).astype(np.float32)
    p = np.arange(128)[:, None]
    c = np.arange(512)[None, :]
    # 0/1 keep-mask for the diagonal key tiles: key p + r*128 <= query c
    maskB = np.concatenate(
        [((p + r * 128) <= c).astype(np.float32) for r in range(4)],
        axis=1).astype(bf)
    ones128 = np.ones((128, 128), dtype=bf)

    def panels_qk(W, g):
        # [h, p, i*128+m] = W_core_T[i*128+p, h*128+m]
        WT = W[g * GD:(g + 1) * GD, :].T  # [C, GD]
        A = WT.reshape(NCT, 128, HG, 128).transpose(2, 1, 0, 3)
        return np.ascontiguousarray(A.reshape(HG, 128, NCT * 128)).astype(bf)

    def panels_v(W, g):
        # [pair, p, i*256+n] = W_core_T[i*128+p, pair*256+n]
        WT = W[g * GD:(g + 1) * GD, :].T
        A = WT.reshape(NCT, 128, 4, 256).transpose(2, 1, 0, 3)
        return np.ascontiguousarray(A.reshape(4, 128, NCT * 256)).astype(bf)

    def panel_o(W, g):
        # [p, (cb*8+hh)*512+c] = W_core_T[hh*128+p, cb*512+c]
        WT = W[:, g * GD:(g + 1) * GD].T  # [GD, C]
        A = WT.reshape(HG, 128, 4, 512).transpose(1, 2, 0, 3)
        return np.ascontiguousarray(A.reshape(128, 4 * HG * 512)).astype(bf)

    in_maps = []
    for core in range(8):
        b, g = core // 2, core % 2
        in_maps.append({
            "xt": np.ascontiguousarray(x[b].T).astype(bf),
            "wq": panels_qk(Wq, g),
            "wk": panels_qk(Wk, g),
            "wv": panels_v(Wv, g),
            "wo": panel_o(Wo, g),
            "womg2": womg2, "b16": b16,
            "freqs": freqs,
            "gq": gqv, "gk": gkv,
            "maskB": maskB, "ones128": ones128,
        })
    return in_maps


def kernel(**inputs) -> np.ndarray:
    if "nc" not in _CACHE:
        _CACHE["nc"] = _build()
    nc = _CACHE["nc"]
    in_maps = _host_prep(inputs)
    res = run_bass_kernel_spmd(nc, in_maps, core_ids=list(range(8)))
    out = np.empty((B, T, C), dtype=np.float32)
    for b in range(B):
        out[b] = res.results[2 * b]["out"] + res.results[2 * b + 1]["out"]
    return out
